# revision 5
# baseline (speedup 1.0000x reference)
"""Complex self-attention on 8 Trainium2 NeuronCores (Bass/Tile).

Model (reference): complex linear q/k/v projections of (x_re, x_im), attention
scores = (Re(q)·Re(k) + Im(q)·Im(k))/sqrt(D), softmax, attn applied to Re(v)
and Im(v), complex output projection. B=2, N=2048, C=1024, H=16, D=64.

Sharding: heads tensor-parallel across 8 cores (2 heads/core, both batches).
  - Projections: stacked-contraction trick ([x_re; x_im], 2C rows) against
    host-combined weights — each complex part is ONE bf16 matmul chain.
  - Software-pipelined phase schedule keeps the PE continuously busy (TRN2
    drops to half clock for 3us after ANY idle gap):
      phase 1: proj(b0)
      phase 2: attention(b0) interleaved with proj(b1)  [exp-bound slots
               filled with projection chains]
      phase 3: attention(b1) interleaved with half of o-proj(b0)
      phase 4: rest of o-proj(b0) under the last A2A, then o-proj(b1)
    The softmax tail of each query tile (den matmuls, reciprocal, scale,
    store) is deferred into the NEXT tile's emission so the PE never waits
    for the exp/DVE pipeline at tile boundaries.
  - Softmax denominator: exp tiles are tree-summed on the idle DVE in bf16,
    then 4 ones-matmuls per query tile (instead of 16) do the partition sum.
  - 1/den via reciprocal_approx_fast (~5x faster, 18-bit accurate).
  - k bias dropped (exact by softmax shift invariance); v bias folded into
    the o-projection bias on the host (softmax rows sum to 1).
  - A2A: batch 0 in one exchange (hidden under phase 3), batch 1 per head so
    only the last 0.5 MB exchange is exposed.
"""

import sys

if "/opt/trn_rl_repo" not in sys.path:
    sys.path.insert(0, "/opt/trn_rl_repo")

from contextlib import ExitStack

import ml_dtypes
import numpy as np

import concourse.mybir as mybir
import concourse.tile as tile
from concourse import bacc
from concourse.bass_utils import run_bass_kernel_spmd

B, N, C = 2, 2048, 1024
H, D = 16, 64
T = B * N  # 4096 tokens total
NCORES = 8
HPC = H // NCORES  # 2 heads per core
TSL = N // NCORES  # 256-token output slice per core PER BATCH
TF = 512  # projection token-chunk (free dim)
KT = 2 * C // 128  # 16 contraction tiles of 128 over [x_re; x_im]
NCH = N // TF  # token chunks per batch (4)
NKP = N // 256  # key-tile PAIRS per query tile (8): each pair = 2 x 128 keys
F32 = mybir.dt.float32
BF16 = mybir.dt.bfloat16
BF = ml_dtypes.bfloat16


def _host_prep(inp):
    """Build the host-side sharded/combined arrays (all matmul inputs bf16,
    blocked so every DMA is contiguous per partition)."""
    x_re = np.asarray(inp["x_re"], dtype=np.float32).reshape(T, C)
    x_im = np.asarray(inp["x_im"], dtype=np.float32).reshape(T, C)
    xT2 = np.concatenate([x_re.T, x_im.T], axis=0).astype(BF)  # [2C, T]
    # blocked: [p, chunk, kt, t] so a chunk load is 16KB contiguous/partition
    xb = np.ascontiguousarray(
        xT2.reshape(KT, 128, B * NCH, TF).transpose(1, 2, 0, 3)
    )

    per_core = []
    for c in range(NCORES):
        d = {}
        h0 = c * HPC
        ch = slice(h0 * D, (h0 + HPC) * D)
        for nm in ("q", "k", "v"):
            Wre = np.asarray(inp[f"{nm}_Wre"], dtype=np.float32)[ch]  # [128, C]
            Wim = np.asarray(inp[f"{nm}_Wim"], dtype=np.float32)[ch]
            bre = np.asarray(inp[f"{nm}_bre"], dtype=np.float32)[ch]
            bim = np.asarray(inp[f"{nm}_bim"], dtype=np.float32)[ch]
            Ws, bs = [], []
            for hh in range(HPC):
                hs = slice(hh * D, (hh + 1) * D)
                wr = np.concatenate([Wre[hs].T, -Wim[hs].T], axis=0)  # [2C, 64]
                wi = np.concatenate([Wim[hs].T, Wre[hs].T], axis=0)
                Ws.append(np.concatenate([wr, wi], axis=1))  # [2C, 128]
                bs.append(np.concatenate([bre[hs] - bim[hs], bre[hs] + bim[hs]]))
            if nm == "v":
                wvb = np.concatenate(Ws, axis=1).astype(BF)  # [2C, 256]
                d["wv"] = np.ascontiguousarray(
                    wvb.reshape(KT, 128, 2 * HPC * D).transpose(1, 0, 2)
                )  # [p, kt, 256]
            else:
                wqk = np.stack(Ws).astype(BF)  # [HPC, 2C, 128]
                d[f"w{nm}"] = np.ascontiguousarray(
                    wqk.reshape(HPC, KT, 128, 128).transpose(2, 0, 1, 3)
                )  # [p, hh, kt, m]
                if nm == "q":
                    d["bq"] = np.ascontiguousarray(np.stack(bs, axis=1))  # [128, HPC]
        per_core.append(d)

    # o-projection combined matrices, rows ordered to match the A2A result:
    # src rank r, then per rank [h0:out_r(64), h0:out_i(64), h1:..., h1:...]
    oWre = np.asarray(inp["o_Wre"], dtype=np.float32)
    oWim = np.asarray(inp["o_Wim"], dtype=np.float32)
    Mre_rows, Mim_rows = [], []
    bv_rows = []
    vbre = np.asarray(inp["v_bre"], dtype=np.float32)
    vbim = np.asarray(inp["v_bim"], dtype=np.float32)
    for r in range(NCORES):
        for hh in range(HPC):
            h = r * HPC + hh
            hs = slice(h * D, (h + 1) * D)
            Mre_rows += [oWre[:, hs].T, -oWim[:, hs].T]
            Mim_rows += [oWim[:, hs].T, oWre[:, hs].T]
            bv_rows += [vbre[hs] - vbim[hs], vbre[hs] + vbim[hs]]
    M_re = np.concatenate(Mre_rows, axis=0)  # [2C, C]
    M_im = np.concatenate(Mim_rows, axis=0)
    bv_full = np.concatenate(bv_rows)  # [2C] — v bias in A2A row order
    o_bre = np.asarray(inp["o_bre"], dtype=np.float32)
    o_bim = np.asarray(inp["o_bim"], dtype=np.float32)
    # fold the v bias through the o-projection (softmax rows sum to 1)
    bo_re = (o_bre - o_bim) + M_re.T @ bv_full  # [C]
    bo_im = (o_bre + o_bim) + M_im.T @ bv_full
    bo_re = np.ascontiguousarray(bo_re.reshape(8, 128).T.astype(np.float32))  # [128, 8]
    bo_im = np.ascontiguousarray(bo_im.reshape(8, 128).T.astype(np.float32))

    def mblk(M):  # [2C, C] -> [p, g, kt, 512] contiguous per partition
        Mb = M.astype(BF).reshape(KT, 128, 2, 512)
        return np.ascontiguousarray(Mb.transpose(1, 2, 0, 3))

    shared = dict(
        xb=xb, M_re=mblk(M_re), M_im=mblk(M_im), bo_re=bo_re, bo_im=bo_im
    )
    return shared, per_core


def _build_program():
    nc = bacc.Bacc("TRN2", target_bir_lowering=False, debug=False, num_devices=NCORES)

    # ---- DRAM I/O (host-blocked layouts: contiguous per partition) ----
    xb_d = nc.dram_tensor("xb", [128, B * NCH, KT, TF], BF16, kind="ExternalInput")
    wq_d = nc.dram_tensor("wq", [128, HPC, KT, 128], BF16, kind="ExternalInput")
    wk_d = nc.dram_tensor("wk", [128, HPC, KT, 128], BF16, kind="ExternalInput")
    wv_d = nc.dram_tensor("wv", [128, KT, 2 * HPC * D], BF16, kind="ExternalInput")
    bq_d = nc.dram_tensor("bq", [128, HPC], F32, kind="ExternalInput")
    Mre_d = nc.dram_tensor("M_re", [128, 2, KT, 512], BF16, kind="ExternalInput")
    Mim_d = nc.dram_tensor("M_im", [128, 2, KT, 512], BF16, kind="ExternalInput")
    bore_d = nc.dram_tensor("bo_re", [128, 8], F32, kind="ExternalInput")
    boim_d = nc.dram_tensor("bo_im", [128, 8], F32, kind="ExternalInput")
    # per-core output: rows = [re(1024); im(1024)], cols = [b0 slice | b1 slice]
    yout_d = nc.dram_tensor("yout", [2 * C, B * TSL], F32, kind="ExternalOutput")

    yout_t = yout_d.rearrange("(cb p) t -> p cb t", p=128)  # [128, 16, 512]

    with (
        tile.TileContext(nc) as tc,
        nc.allow_low_precision(
            reason="bf16 intermediates; rounding matches low-precision matmul noise"
        ),
    ):
        with tc.tile_pool(name="dram", bufs=1, space="DRAM") as dram:
            # A2A buffers: [dest rank, ch, TSL tok]; batch 0 whole, b1 per head
            outc_dr0 = dram.tile([NCORES, 256, TSL], BF16, name="outc0", tag="outc0")
            at_dr0 = dram.tile([NCORES, 256, TSL], BF16, name="at0d", tag="at0d")
            outc_dr1 = [
                dram.tile([NCORES, 128, TSL], BF16, name=f"outc1{h}", tag=f"outc1{h}")
                for h in range(HPC)
            ]
            at_dr1 = [
                dram.tile([NCORES, 128, TSL], BF16, name=f"at1{h}", tag=f"at1{h}")
                for h in range(HPC)
            ]

            with (
                tc.tile_pool(name="keep", bufs=1) as keep,
                # PSUM: pair(2 banks x2) + av(1 bank x2) + den(1 bank x2) = 8.
                tc.tile_pool(name="pair_ps", bufs=2, space="PSUM") as pair_ps,
                tc.tile_pool(name="av_ps", bufs=2, space="PSUM") as av_ps,
                tc.tile_pool(name="den_ps", bufs=2, space="PSUM") as den_ps,
                tc.tile_pool(name="qk_sb", bufs=1) as qk_sb,
                tc.tile_pool(name="v_sb", bufs=1) as v_sbp,
                tc.tile_pool(name="expp", bufs=4) as expp,
                tc.tile_pool(name="dtp", bufs=2) as dtp,
                tc.tile_pool(name="evp", bufs=3) as evp,
                tc.tile_pool(name="const", bufs=1) as const,
            ):
                ctx_x = ExitStack()
                xp = ctx_x.enter_context(tc.tile_pool(name="xp", bufs=2))

                # ---- startup: first x chunk split per-2kt across BOTH data
                # rings; weights per-kt on the gpsimd ring so the first chain
                # starts ~2us in ----
                xtiles = {}
                xt0 = xp.tile([128, KT, TF], BF16, name="xt", tag="xt")
                xtiles[0] = xt0
                for ks in range(0, KT, 2):
                    eng = nc.sync if (ks // 2) % 2 == 0 else nc.scalar
                    eng.dma_start(xt0[:, ks : ks + 2, :], xb_d[:, 0, ks : ks + 2, :])
                wq_sb = const.tile([128, HPC, KT, 128], BF16)
                wk_sb = const.tile([128, HPC, KT, 128], BF16)
                wv_sb = const.tile([128, KT, 2 * HPC * D], BF16)
                for kt in range(KT):
                    nc.gpsimd.dma_start(wq_sb[:, :, kt, :], wq_d[:, :, kt, :])
                nc.gpsimd.dma_start(wk_sb[:], wk_d[:])
                nc.gpsimd.dma_start(wv_sb[:], wv_d[:])
                bq_sb = keep.tile([128, HPC], F32)
                nc.gpsimd.dma_start(bq_sb[:], bq_d[:])
                ones16 = keep.tile([128, 128], BF16)
                nc.vector.memset(ones16[:], 1.0)

                qc = {}
                kc = {}
                vt = {}
                for b in range(B):
                    for hh in range(HPC):
                        qc[b, hh] = qk_sb.tile(
                            [128, N], BF16, name=f"qc{b}{hh}", tag=f"qc{b}{hh}"
                        )
                        kc[b, hh] = qk_sb.tile(
                            [128, N], BF16, name=f"kc{b}{hh}", tag=f"kc{b}{hh}"
                        )
                        vt[b, hh] = v_sbp.tile(
                            [128, N // 128, 128],
                            BF16,
                            name=f"vt{b}{hh}",
                            tag=f"vt{b}{hh}",
                        )

                def emit_x_dma(cg):
                    """Prefetch x chunk cg (global index)."""
                    xt = xp.tile([128, KT, TF], BF16, name="xt", tag="xt")
                    xtiles[cg] = xt
                    eng = nc.scalar if cg % 2 == 0 else nc.sync
                    eng.dma_start(xt[:], xb_d[:, cg, :, :])

                def emit_proj_qk(b, ci):
                    """q and k chains for chunk ci of batch b."""
                    cg = b * NCH + ci
                    xt = xtiles[cg]
                    csl = slice(ci * TF, ci * TF + TF)
                    prs = {}
                    for hh in range(HPC):
                        prs[hh] = pair_ps.tile([128, 1024], F32, name="prps", tag="prps")
                        for kt in range(KT):
                            nc.tensor.matmul(
                                prs[hh][:, 0:512],
                                wq_sb[:, hh, kt, :],
                                xt[:, kt, :],
                                start=(kt == 0),
                                stop=(kt == KT - 1),
                            )
                        nc.scalar.activation(
                            qc[b, hh][:, csl],
                            prs[hh][:, 0:512],
                            mybir.ActivationFunctionType.Identity,
                            bias=bq_sb[:, hh : hh + 1],
                        )
                    for hh in range(HPC):
                        for kt in range(KT):
                            nc.tensor.matmul(
                                prs[hh][:, 512:1024],
                                wk_sb[:, hh, kt, :],
                                xt[:, kt, :],
                                start=(kt == 0),
                                stop=(kt == KT - 1),
                            )
                        nc.vector.tensor_copy(kc[b, hh][:, csl], prs[hh][:, 512:1024])

                def emit_proj_v(b, ci):
                    """v chains for chunk ci of batch b."""
                    cg = b * NCH + ci
                    xt = xtiles[cg]
                    vtiles = [
                        pair_ps.tile([128, 1024], F32, name="prps", tag="prps")
                        for _ in range(2)
                    ]
                    for m in range(TF // 128):
                        # alternate tiles and half-banks so the DVE read of
                        # chain m-1 never shares a bank with chain m's write
                        vp = vtiles[m % 2][:, (m // 2) * 512 : (m // 2) * 512 + 256]
                        for kt in range(KT):
                            nc.tensor.matmul(
                                vp,
                                xt[:, kt, m * 128 : (m + 1) * 128],
                                wv_sb[:, kt, :],
                                start=(kt == 0),
                                stop=(kt == KT - 1),
                            )
                        ktok = ci * (TF // 128) + m
                        for hh in range(HPC):
                            nc.vector.tensor_copy(
                                vt[b, hh][:, ktok, :],
                                vp[:, hh * 128 : (hh + 1) * 128],
                            )

                deferred = [None]  # single-slot pipeline for softmax tails

                def flush_deferred():
                    if deferred[0] is not None:
                        deferred[0]()
                        deferred[0] = None

                def emit_attn_qt(b, hh, qt):
                    """Scores/exp/av for one 512-query tile; the softmax tail
                    (den matmuls, reciprocal, scale, store) is deferred into
                    the next tile so the PE never idles at tile boundaries."""
                    qsl = slice(qt * 512, (qt + 1) * 512)
                    av = av_ps.tile([128, 512], F32, name="avps", tag="avps")

                    def _av(kp, ex):
                        for j in range(2):
                            kt = 2 * kp + j
                            nc.tensor.matmul(
                                av[:],
                                vt[b, hh][:, kt, :],
                                ex[:, j * 512 : (j + 1) * 512],
                                start=(kt == 0),
                                stop=(kt == N // 128 - 1),
                            )

                    pend = []
                    exs = []
                    pairs = {}
                    quads = {}
                    for kp in range(NKP):
                        pr = pair_ps.tile([128, 1024], F32, name="prps", tag="prps")
                        for j in range(2):
                            kt = 2 * kp + j
                            nc.tensor.matmul(
                                pr[:, j * 512 : (j + 1) * 512],
                                kc[b, hh][:, kt * 128 : (kt + 1) * 128],
                                qc[b, hh][:, qsl],
                                start=True,
                                stop=True,
                            )
                        ex = expp.tile([128, 1024], BF16, name="ex", tag="ex")
                        nc.scalar.activation(
                            ex[:], pr[:], mybir.ActivationFunctionType.Exp, scale=0.125
                        )
                        exs.append(ex)
                        # denominator tree-adds on the idle DVE (bf16 2x mode)
                        if kp % 2 == 1:
                            p = kp // 2
                            pairs[p] = dtp.tile(
                                [128, 1024], BF16, name=f"dp{p % 2}", tag=f"dp{p % 2}"
                            )
                            nc.vector.tensor_tensor(
                                pairs[p][:], exs[kp - 1][:], ex[:], mybir.AluOpType.add
                            )
                        if kp % 4 == 3:
                            q4 = kp // 4
                            quads[q4] = dtp.tile(
                                [128, 1024], BF16, name=f"dq{q4}", tag=f"dq{q4}"
                            )
                            nc.vector.tensor_tensor(
                                quads[q4][:],
                                pairs[kp // 2 - 1][:],
                                pairs[kp // 2][:],
                                mybir.AluOpType.add,
                            )
                        pend.append((kp, ex))
                        if kp == 1:
                            # PE is 2 key-pairs into this tile: safe point to
                            # emit the previous tile's softmax tail
                            flush_deferred()
                        if len(pend) > 2:
                            _av(*pend.pop(0))
                    pend_tail = list(pend)

                    def tail():
                        # last two av accumulations land here so the PE never
                        # waits on the exp pipeline at the tile boundary
                        for item in pend_tail:
                            _av(*item)
                        den = den_ps.tile([128, 512], F32, name="denps", tag="denps")
                        for i4, q4t in enumerate((quads[0], quads[1])):
                            for j in range(2):
                                nc.tensor.matmul(
                                    den[:],
                                    ones16[:],
                                    q4t[:, j * 512 : (j + 1) * 512],
                                    start=(i4 == 0 and j == 0),
                                    stop=(i4 == 1 and j == 1),
                                )
                        rb = evp.tile([128, 512], F32, name="rb", tag="rb")
                        nc.vector.reciprocal_approx_fast(rb[:], den[:])
                        outc = evp.tile([128, 512], BF16, name="outc", tag="outc")
                        nc.vector.tensor_tensor(
                            outc[:], av[:], rb[:], mybir.AluOpType.mult
                        )
                        for j in range(2):
                            if b == 0:
                                dst = outc_dr0[2 * qt + j, hh * 128 : (hh + 1) * 128, :]
                            else:
                                dst = outc_dr1[hh][2 * qt + j, :, :]
                            nc.sync.dma_start(dst, outc[:, j * TSL : (j + 1) * TSL])

                    deferred[0] = tail

                # ---- phase 1: projections for batch 0 ----
                for ci in range(NCH):
                    if ci + 1 < NCH:
                        emit_x_dma(ci + 1)
                    emit_proj_qk(0, ci)
                    emit_proj_v(0, ci)
                emit_x_dma(NCH)  # first b1 chunk

                # ---- phase 2: attention(b0) interleaved with proj(b1) ----
                for hh in range(HPC):
                    for qt in range(N // 512):
                        emit_attn_qt(0, hh, qt)
                        si = hh * 4 + qt
                        ci = si // 2
                        if si % 2 == 0:
                            if ci + 1 < NCH:
                                emit_x_dma(NCH + ci + 1)
                            emit_proj_qk(1, ci)
                        else:
                            emit_proj_v(1, ci)
                flush_deferred()
                nc.gpsimd.collective_compute(
                    "AllToAll",
                    mybir.AluOpType.bypass,
                    replica_groups=[list(range(NCORES))],
                    ins=[outc_dr0.opt()],
                    outs=[at_dr0.opt()],
                )

                # x pool no longer needed: free it for the o-phase M tiles
                ctx_x.close()
                opool = ctx_x.enter_context(tc.tile_pool(name="opool", bufs=1))
                oev = ctx_x.enter_context(tc.tile_pool(name="oev", bufs=2))
                m_tiles = []
                for g in range(2):
                    for part, M_d in ((0, Mre_d), (1, Mim_d)):
                        m_sb = opool.tile(
                            [128, KT, 512], BF16, name=f"m{g}{part}", tag=f"m{g}{part}"
                        )
                        eng = nc.scalar if part == 0 else nc.sync
                        eng.dma_start(m_sb[:], M_d[:, g, :, :])
                        m_tiles.append((g, part, m_sb))
                bo_sb = [None, None]
                bo_sb[0] = keep.tile([128, 8], F32, name="bo_re", tag="bo_re")
                bo_sb[1] = keep.tile([128, 8], F32, name="bo_im", tag="bo_im")
                nc.gpsimd.dma_start(bo_sb[0][:], bore_d[:])
                nc.gpsimd.dma_start(bo_sb[1][:], boim_d[:])
                at_sb = [None, None]
                at_sb[0] = opool.tile(
                    [128, HPC, NCORES, TSL], BF16, name="at0", tag="at0"
                )
                at_sb[1] = opool.tile(
                    [128, HPC, NCORES, TSL], BF16, name="at1", tag="at1"
                )
                # at0 load on the gpsimd queue right after the A2A it waits on
                at0_t = at_dr0.rearrange("r (hp p) t -> p hp r t", p=128)
                for h in range(HPC):  # DMA APs are limited to 3 dims
                    nc.gpsimd.dma_start(at_sb[0][:, h, :, :], at0_t[:, h, :, :])

                def emit_oproj_group(b, g, part, m_sb):
                    """One quarter of the o-projection for batch b: 4 chains
                    of 128 output channels x TSL tokens + bias + store."""
                    otiles = [
                        pair_ps.tile([128, 1024], F32, name="prps", tag="prps"),
                        pair_ps.tile([128, 1024], F32, name="prps", tag="prps"),
                    ]
                    y_sb = oev.tile([128, 4, TSL], F32, name="y_sb", tag="y_sb")
                    for i in range(4):
                        # alternate tiles/half-banks: ACT reads chain i-1's
                        # bank while the PE accumulates into another
                        ps = otiles[i % 2][:, (i // 2) * 512 : (i // 2) * 512 + TSL]
                        for kt in range(KT):
                            # contraction row kt = (src rank kt//2, head kt%2)
                            nc.tensor.matmul(
                                ps,
                                m_sb[:, kt, i * 128 : (i + 1) * 128],
                                at_sb[b][:, kt % 2, kt // 2, :],
                                start=(kt == 0),
                                stop=(kt == KT - 1),
                            )
                        nc.scalar.activation(
                            y_sb[:, i, :],
                            ps,
                            mybir.ActivationFunctionType.Identity,
                            bias=bo_sb[part][:, g * 4 + i : g * 4 + i + 1],
                        )
                    cb0 = part * 8 + g * 4
                    nc.sync.dma_start(
                        yout_t[:, cb0 : cb0 + 4, b * TSL : (b + 1) * TSL], y_sb[:]
                    )

                # ---- phase 3: attention(b1) interleaved with o-proj(b0) ----
                for hh in range(HPC):
                    for qt in range(N // 512):
                        emit_attn_qt(1, hh, qt)
                        if hh == 1 and qt in (0, 2):
                            gi = qt // 2
                            emit_oproj_group(0, *m_tiles[gi][:2], m_tiles[gi][2])
                    flush_deferred()
                    # head hh's rows ship while the next head computes
                    nc.gpsimd.collective_compute(
                        "AllToAll",
                        mybir.AluOpType.bypass,
                        replica_groups=[list(range(NCORES))],
                        ins=[outc_dr1[hh].opt()],
                        outs=[at_dr1[hh].opt()],
                    )

                # ---- phase 4: rest of o-proj(b0) under the last A2A ----
                for gi in (2, 3):
                    emit_oproj_group(0, *m_tiles[gi][:2], m_tiles[gi][2])
                for h in range(HPC):
                    at1h_t = at_dr1[h].rearrange("r p t -> p r t")
                    nc.gpsimd.dma_start(at_sb[1][:, h, :, :], at1h_t[:, :, :])
                for gi in range(4):
                    emit_oproj_group(1, *m_tiles[gi][:2], m_tiles[gi][2])
                ctx_x.close()  # opool/oev close before the outer pools (LIFO)
    nc.compile()
    return nc


_NC_CACHE = None


def _get_program():
    global _NC_CACHE
    if _NC_CACHE is None:
        _NC_CACHE = _build_program()
    return _NC_CACHE


def _run(inputs, trace=False, trace_kwargs=None):
    shared, per_core = _host_prep(inputs)
    nc = _get_program()
    in_maps = []
    for c in range(NCORES):
        d = per_core[c]
        in_maps.append(
            {
                "xb": shared["xb"],
                "wq": d["wq"],
                "wk": d["wk"],
                "wv": d["wv"],
                "bq": d["bq"],
                "M_re": shared["M_re"],
                "M_im": shared["M_im"],
                "bo_re": shared["bo_re"],
                "bo_im": shared["bo_im"],
            }
        )
    res = run_bass_kernel_spmd(
        nc, in_maps, list(range(NCORES)), trace=trace, **(trace_kwargs or {})
    )
    youts = [res.results[c]["yout"] for c in range(NCORES)]
    # youts[c]: [2C, B*TSL]; rows [re(1024); im(1024)], cols [b0 256 | b1 256]
    re = np.zeros((B, N, C), dtype=np.float32)
    im = np.zeros((B, N, C), dtype=np.float32)
    for c in range(NCORES):
        for b in range(B):
            tsl = slice(c * TSL, (c + 1) * TSL)
            re[b, tsl] = youts[c][:C, b * TSL : (b + 1) * TSL].T
            im[b, tsl] = youts[c][C:, b * TSL : (b + 1) * TSL].T
    return np.stack([re, im]).astype(np.float32), res


def kernel(**inputs) -> np.ndarray:
    out, _ = _run(inputs, trace=False)
    return out


# revision 7
# speedup vs baseline: 1.0186x; 1.0186x over previous
"""Complex self-attention on 8 Trainium2 NeuronCores (Bass/Tile).

Model (reference): complex linear q/k/v projections of (x_re, x_im), attention
scores = (Re(q)·Re(k) + Im(q)·Im(k))/sqrt(D), softmax, attn applied to Re(v)
and Im(v), complex output projection. B=2, N=2048, C=1024, H=16, D=64.

Sharding: heads tensor-parallel across 8 cores (2 heads/core, both batches).
  - Projections: stacked-contraction trick ([x_re; x_im], 2C rows) against
    host-combined weights — each complex part is ONE bf16 matmul chain.
  - Software-pipelined phase schedule keeps the PE continuously busy (TRN2
    drops to half clock for 3us after ANY idle gap):
      phase 1: proj(b0)
      phase 2: attention(b0) interleaved with proj(b1)  [exp-bound slots
               filled with projection chains]
      phase 3: attention(b1) interleaved with half of o-proj(b0)
      phase 4: rest of o-proj(b0) under the last A2A, then o-proj(b1)
    The softmax tail of each query tile (den matmuls, reciprocal, scale,
    store) is deferred into the NEXT tile's emission so the PE never waits
    for the exp/DVE pipeline at tile boundaries.
  - Softmax denominator: exp tiles are tree-summed on the idle DVE in bf16,
    then 4 ones-matmuls per query tile (instead of 16) do the partition sum.
  - 1/den via reciprocal_approx_fast (~5x faster, 18-bit accurate).
  - k bias dropped (exact by softmax shift invariance); v bias folded into
    the o-projection bias on the host (softmax rows sum to 1).
  - A2A: batch 0 in one exchange (hidden under phase 3), batch 1 per head so
    only the last 0.5 MB exchange is exposed.
"""

import sys

if "/opt/trn_rl_repo" not in sys.path:
    sys.path.insert(0, "/opt/trn_rl_repo")

from contextlib import ExitStack

import ml_dtypes
import numpy as np

import concourse.mybir as mybir
import concourse.tile as tile
from concourse import bacc
from concourse.bass_utils import run_bass_kernel_spmd

B, N, C = 2, 2048, 1024
H, D = 16, 64
T = B * N  # 4096 tokens total
NCORES = 8
HPC = H // NCORES  # 2 heads per core
TSL = N // NCORES  # 256-token output slice per core PER BATCH
TF = 512  # projection token-chunk (free dim)
KT = 2 * C // 128  # 16 contraction tiles of 128 over [x_re; x_im]
NCH = N // TF  # token chunks per batch (4)
NKP = N // 256  # key-tile PAIRS per query tile (8): each pair = 2 x 128 keys
F32 = mybir.dt.float32
BF16 = mybir.dt.bfloat16
BF = ml_dtypes.bfloat16


def _host_prep(inp):
    """Build the host-side sharded/combined arrays (all matmul inputs bf16,
    blocked so every DMA is contiguous per partition)."""
    x_re = np.asarray(inp["x_re"], dtype=np.float32).reshape(T, C)
    x_im = np.asarray(inp["x_im"], dtype=np.float32).reshape(T, C)
    xT2 = np.concatenate([x_re.T, x_im.T], axis=0).astype(BF)  # [2C, T]
    # blocked: [p, chunk, kt, t] so a chunk load is 16KB contiguous/partition
    xb = np.ascontiguousarray(
        xT2.reshape(KT, 128, B * NCH, TF).transpose(1, 2, 0, 3)
    )

    per_core = []
    for c in range(NCORES):
        d = {}
        h0 = c * HPC
        ch = slice(h0 * D, (h0 + HPC) * D)
        for nm in ("q", "k", "v"):
            Wre = np.asarray(inp[f"{nm}_Wre"], dtype=np.float32)[ch]  # [128, C]
            Wim = np.asarray(inp[f"{nm}_Wim"], dtype=np.float32)[ch]
            bre = np.asarray(inp[f"{nm}_bre"], dtype=np.float32)[ch]
            bim = np.asarray(inp[f"{nm}_bim"], dtype=np.float32)[ch]
            Ws, bs = [], []
            for hh in range(HPC):
                hs = slice(hh * D, (hh + 1) * D)
                wr = np.concatenate([Wre[hs].T, -Wim[hs].T], axis=0)  # [2C, 64]
                wi = np.concatenate([Wim[hs].T, Wre[hs].T], axis=0)
                Ws.append(np.concatenate([wr, wi], axis=1))  # [2C, 128]
                bs.append(np.concatenate([bre[hs] - bim[hs], bre[hs] + bim[hs]]))
            if nm == "v":
                wvb = np.concatenate(Ws, axis=1).astype(BF)  # [2C, 256]
                d["wv"] = np.ascontiguousarray(
                    wvb.reshape(KT, 128, 2 * HPC * D).transpose(1, 0, 2)
                )  # [p, kt, 256]
            else:
                wqk = np.stack(Ws).astype(BF)  # [HPC, 2C, 128]
                d[f"w{nm}"] = np.ascontiguousarray(
                    wqk.reshape(HPC, KT, 128, 128).transpose(2, 0, 1, 3)
                )  # [p, hh, kt, m]
                if nm == "q":
                    d["bq"] = np.ascontiguousarray(np.stack(bs, axis=1))  # [128, HPC]
        per_core.append(d)

    # o-projection combined matrices, rows ordered to match the A2A result:
    # src rank r, then per rank [h0:out_r(64), h0:out_i(64), h1:..., h1:...]
    oWre = np.asarray(inp["o_Wre"], dtype=np.float32)
    oWim = np.asarray(inp["o_Wim"], dtype=np.float32)
    Mre_rows, Mim_rows = [], []
    bv_rows = []
    vbre = np.asarray(inp["v_bre"], dtype=np.float32)
    vbim = np.asarray(inp["v_bim"], dtype=np.float32)
    for r in range(NCORES):
        for hh in range(HPC):
            h = r * HPC + hh
            hs = slice(h * D, (h + 1) * D)
            Mre_rows += [oWre[:, hs].T, -oWim[:, hs].T]
            Mim_rows += [oWim[:, hs].T, oWre[:, hs].T]
            bv_rows += [vbre[hs] - vbim[hs], vbre[hs] + vbim[hs]]
    M_re = np.concatenate(Mre_rows, axis=0)  # [2C, C]
    M_im = np.concatenate(Mim_rows, axis=0)
    bv_full = np.concatenate(bv_rows)  # [2C] — v bias in A2A row order
    o_bre = np.asarray(inp["o_bre"], dtype=np.float32)
    o_bim = np.asarray(inp["o_bim"], dtype=np.float32)
    # fold the v bias through the o-projection (softmax rows sum to 1)
    bo_re = (o_bre - o_bim) + M_re.T @ bv_full  # [C]
    bo_im = (o_bre + o_bim) + M_im.T @ bv_full
    bo_re = np.ascontiguousarray(bo_re.reshape(8, 128).T.astype(np.float32))  # [128, 8]
    bo_im = np.ascontiguousarray(bo_im.reshape(8, 128).T.astype(np.float32))

    def mblk(M):  # [2C, C] -> [p, g, kt, 512] contiguous per partition
        Mb = M.astype(BF).reshape(KT, 128, 2, 512)
        return np.ascontiguousarray(Mb.transpose(1, 2, 0, 3))

    shared = dict(
        xb=xb, M_re=mblk(M_re), M_im=mblk(M_im), bo_re=bo_re, bo_im=bo_im
    )
    return shared, per_core


def _build_program():
    nc = bacc.Bacc("TRN2", target_bir_lowering=False, debug=False, num_devices=NCORES)

    # ---- DRAM I/O (host-blocked layouts: contiguous per partition) ----
    xb_d = nc.dram_tensor("xb", [128, B * NCH, KT, TF], BF16, kind="ExternalInput")
    wq_d = nc.dram_tensor("wq", [128, HPC, KT, 128], BF16, kind="ExternalInput")
    wk_d = nc.dram_tensor("wk", [128, HPC, KT, 128], BF16, kind="ExternalInput")
    wv_d = nc.dram_tensor("wv", [128, KT, 2 * HPC * D], BF16, kind="ExternalInput")
    bq_d = nc.dram_tensor("bq", [128, HPC], F32, kind="ExternalInput")
    Mre_d = nc.dram_tensor("M_re", [128, 2, KT, 512], BF16, kind="ExternalInput")
    Mim_d = nc.dram_tensor("M_im", [128, 2, KT, 512], BF16, kind="ExternalInput")
    bore_d = nc.dram_tensor("bo_re", [128, 8], F32, kind="ExternalInput")
    boim_d = nc.dram_tensor("bo_im", [128, 8], F32, kind="ExternalInput")
    # per-core output: rows = [re(1024); im(1024)], cols = [b0 slice | b1 slice]
    yout_d = nc.dram_tensor("yout", [2 * C, B * TSL], F32, kind="ExternalOutput")

    yout_t = yout_d.rearrange("(cb p) t -> p cb t", p=128)  # [128, 16, 512]

    with (
        tile.TileContext(nc) as tc,
        nc.allow_low_precision(
            reason="bf16 intermediates; rounding matches low-precision matmul noise"
        ),
    ):
        with tc.tile_pool(name="dram", bufs=1, space="DRAM") as dram:
            # A2A buffers: [dest rank, ch, TSL tok]; batch 0 whole, b1 per head
            outc_dr0 = dram.tile([NCORES, 256, TSL], BF16, name="outc0", tag="outc0")
            at_dr0 = dram.tile([NCORES, 256, TSL], BF16, name="at0d", tag="at0d")
            outc_dr1 = [
                dram.tile([NCORES, 128, TSL], BF16, name=f"outc1{h}", tag=f"outc1{h}")
                for h in range(HPC)
            ]
            at_dr1 = [
                dram.tile([NCORES, 128, TSL], BF16, name=f"at1{h}", tag=f"at1{h}")
                for h in range(HPC)
            ]

            with (
                tc.tile_pool(name="keep", bufs=1) as keep,
                # PSUM: pair(2 banks x2) + av(1 bank x2) + den(1 bank x2) = 8.
                tc.tile_pool(name="pair_ps", bufs=2, space="PSUM") as pair_ps,
                tc.tile_pool(name="av_ps", bufs=2, space="PSUM") as av_ps,
                tc.tile_pool(name="den_ps", bufs=2, space="PSUM") as den_ps,
                tc.tile_pool(name="qk_sb", bufs=1) as qk_sb,
                tc.tile_pool(name="v_sb", bufs=1) as v_sbp,
                tc.tile_pool(name="expp", bufs=4) as expp,
                tc.tile_pool(name="dtp", bufs=2) as dtp,
                tc.tile_pool(name="evp", bufs=3) as evp,
                tc.tile_pool(name="const", bufs=1) as const,
            ):
                ctx_x = ExitStack()
                xp = ctx_x.enter_context(tc.tile_pool(name="xp", bufs=2))

                # ---- startup: first x chunk split per-2kt across BOTH data
                # rings; weights per-kt on the gpsimd ring so the first chain
                # starts ~2us in ----
                xtiles = {}
                xt0 = xp.tile([128, KT, TF], BF16, name="xt", tag="xt")
                xtiles[0] = xt0
                for ks in range(0, KT, 2):
                    eng = nc.sync if (ks // 2) % 2 == 0 else nc.scalar
                    eng.dma_start(xt0[:, ks : ks + 2, :], xb_d[:, 0, ks : ks + 2, :])
                wq_sb = const.tile([128, HPC, KT, 128], BF16)
                wk_sb = const.tile([128, HPC, KT, 128], BF16)
                wv_sb = const.tile([128, KT, 2 * HPC * D], BF16)
                for kt in range(KT):
                    nc.gpsimd.dma_start(wq_sb[:, :, kt, :], wq_d[:, :, kt, :])
                nc.gpsimd.dma_start(wk_sb[:], wk_d[:])
                nc.gpsimd.dma_start(wv_sb[:], wv_d[:])
                bq_sb = keep.tile([128, HPC], F32)
                nc.gpsimd.dma_start(bq_sb[:], bq_d[:])
                ones16 = keep.tile([128, 128], BF16)
                nc.vector.memset(ones16[:], 1.0)

                qc = {}
                kc = {}
                vt = {}
                for b in range(B):
                    for hh in range(HPC):
                        qc[b, hh] = qk_sb.tile(
                            [128, N], BF16, name=f"qc{b}{hh}", tag=f"qc{b}{hh}"
                        )
                        kc[b, hh] = qk_sb.tile(
                            [128, N], BF16, name=f"kc{b}{hh}", tag=f"kc{b}{hh}"
                        )
                        vt[b, hh] = v_sbp.tile(
                            [128, N // 128, 128],
                            BF16,
                            name=f"vt{b}{hh}",
                            tag=f"vt{b}{hh}",
                        )

                def emit_x_dma(cg):
                    """Prefetch x chunk cg (global index)."""
                    xt = xp.tile([128, KT, TF], BF16, name="xt", tag="xt")
                    xtiles[cg] = xt
                    eng = nc.scalar if cg % 2 == 0 else nc.sync
                    eng.dma_start(xt[:], xb_d[:, cg, :, :])

                def emit_proj_qk(b, ci):
                    """q and k chains for chunk ci of batch b."""
                    cg = b * NCH + ci
                    xt = xtiles[cg]
                    csl = slice(ci * TF, ci * TF + TF)
                    prs = {}
                    for hh in range(HPC):
                        prs[hh] = pair_ps.tile([128, 1024], F32, name="prps", tag="prps")
                        for kt in range(KT):
                            nc.tensor.matmul(
                                prs[hh][:, 0:512],
                                wq_sb[:, hh, kt, :],
                                xt[:, kt, :],
                                start=(kt == 0),
                                stop=(kt == KT - 1),
                            )
                        nc.scalar.activation(
                            qc[b, hh][:, csl],
                            prs[hh][:, 0:512],
                            mybir.ActivationFunctionType.Identity,
                            bias=bq_sb[:, hh : hh + 1],
                        )
                    for hh in range(HPC):
                        for kt in range(KT):
                            nc.tensor.matmul(
                                prs[hh][:, 512:1024],
                                wk_sb[:, hh, kt, :],
                                xt[:, kt, :],
                                start=(kt == 0),
                                stop=(kt == KT - 1),
                            )
                        nc.vector.tensor_copy(kc[b, hh][:, csl], prs[hh][:, 512:1024])

                def emit_proj_v(b, ci):
                    """v chains for chunk ci of batch b."""
                    cg = b * NCH + ci
                    xt = xtiles[cg]
                    vtiles = [
                        pair_ps.tile([128, 1024], F32, name="prps", tag="prps")
                        for _ in range(2)
                    ]
                    for m in range(TF // 128):
                        # alternate tiles and half-banks so the DVE read of
                        # chain m-1 never shares a bank with chain m's write
                        vp = vtiles[m % 2][:, (m // 2) * 512 : (m // 2) * 512 + 256]
                        for kt in range(KT):
                            nc.tensor.matmul(
                                vp,
                                xt[:, kt, m * 128 : (m + 1) * 128],
                                wv_sb[:, kt, :],
                                start=(kt == 0),
                                stop=(kt == KT - 1),
                            )
                        ktok = ci * (TF // 128) + m
                        for hh in range(HPC):
                            nc.vector.tensor_copy(
                                vt[b, hh][:, ktok, :],
                                vp[:, hh * 128 : (hh + 1) * 128],
                            )

                deferred = [None]  # single-slot pipeline for softmax tails

                def flush_deferred():
                    if deferred[0] is not None:
                        deferred[0]()
                        deferred[0] = None

                def emit_attn_qt(b, hh, qt):
                    """Scores/exp/av for one 512-query tile; the softmax tail
                    (den matmuls, reciprocal, scale, store) is deferred into
                    the next tile so the PE never idles at tile boundaries."""
                    qsl = slice(qt * 512, (qt + 1) * 512)
                    av = av_ps.tile([128, 512], F32, name="avps", tag="avps")

                    def _av(kp, ex):
                        for j in range(2):
                            kt = 2 * kp + j
                            nc.tensor.matmul(
                                av[:],
                                vt[b, hh][:, kt, :],
                                ex[:, j * 512 : (j + 1) * 512],
                                start=(kt == 0),
                                stop=(kt == N // 128 - 1),
                            )

                    pend = []
                    exs = []
                    pairs = {}
                    quads = {}
                    for kp in range(NKP):
                        pr = pair_ps.tile([128, 1024], F32, name="prps", tag="prps")
                        for j in range(2):
                            kt = 2 * kp + j
                            nc.tensor.matmul(
                                pr[:, j * 512 : (j + 1) * 512],
                                kc[b, hh][:, kt * 128 : (kt + 1) * 128],
                                qc[b, hh][:, qsl],
                                start=True,
                                stop=True,
                            )
                        ex = expp.tile([128, 1024], BF16, name="ex", tag="ex")
                        nc.scalar.activation(
                            ex[:], pr[:], mybir.ActivationFunctionType.Exp, scale=0.125
                        )
                        exs.append(ex)
                        # denominator tree-adds on the idle DVE (bf16 2x mode)
                        if kp % 2 == 1:
                            p = kp // 2
                            pairs[p] = dtp.tile(
                                [128, 1024], BF16, name=f"dp{p % 2}", tag=f"dp{p % 2}"
                            )
                            nc.vector.tensor_tensor(
                                pairs[p][:], exs[kp - 1][:], ex[:], mybir.AluOpType.add
                            )
                        if kp % 4 == 3:
                            q4 = kp // 4
                            quads[q4] = dtp.tile(
                                [128, 1024], BF16, name=f"dq{q4}", tag=f"dq{q4}"
                            )
                            nc.vector.tensor_tensor(
                                quads[q4][:],
                                pairs[kp // 2 - 1][:],
                                pairs[kp // 2][:],
                                mybir.AluOpType.add,
                            )
                        pend.append((kp, ex))
                        if kp == 1:
                            # PE is 2 key-pairs into this tile: safe point to
                            # emit the previous tile's softmax tail
                            flush_deferred()
                        if len(pend) > 2:
                            _av(*pend.pop(0))
                    pend_tail = list(pend)

                    def tail():
                        # last two av accumulations land here so the PE never
                        # waits on the exp pipeline at the tile boundary
                        for item in pend_tail:
                            _av(*item)
                        den = den_ps.tile([128, 512], F32, name="denps", tag="denps")
                        for i4, q4t in enumerate((quads[0], quads[1])):
                            for j in range(2):
                                nc.tensor.matmul(
                                    den[:],
                                    ones16[:],
                                    q4t[:, j * 512 : (j + 1) * 512],
                                    start=(i4 == 0 and j == 0),
                                    stop=(i4 == 1 and j == 1),
                                )
                        rb = evp.tile([128, 512], F32, name="rb", tag="rb")
                        nc.vector.reciprocal_approx_fast(rb[:], den[:])
                        outc = evp.tile([128, 512], BF16, name="outc", tag="outc")
                        nc.vector.tensor_tensor(
                            outc[:], av[:], rb[:], mybir.AluOpType.mult
                        )
                        for j in range(2):
                            if b == 0:
                                dst = outc_dr0[2 * qt + j, hh * 128 : (hh + 1) * 128, :]
                            else:
                                dst = outc_dr1[hh][2 * qt + j, :, :]
                            nc.sync.dma_start(dst, outc[:, j * TSL : (j + 1) * TSL])

                    deferred[0] = tail

                # ---- phase 1: projections for batch 0 ----
                for ci in range(NCH):
                    if ci + 1 < NCH:
                        emit_x_dma(ci + 1)
                    emit_proj_qk(0, ci)
                    emit_proj_v(0, ci)
                emit_x_dma(NCH)  # first two b1 chunks prefetch here
                emit_x_dma(NCH + 1)

                # ---- phase 2a: attention(b0) with HALF of proj(b1) woven in
                # (fills the exp-bound slack and keeps the PE at full clock);
                # the other half runs after the b0 A2A fires, hiding the
                # first collective's cross-core skew sync (~17us) ----
                for hh in range(HPC):
                    for qt in range(N // 512):
                        emit_attn_qt(0, hh, qt)
                        si = hh * 4 + qt
                        if si == 0:
                            emit_proj_qk(1, 0)
                        elif si == 1:
                            emit_proj_v(1, 0)
                        elif si == 2:
                            emit_proj_qk(1, 1)
                        elif si == 3:
                            emit_proj_v(1, 1)
                        elif si == 4:
                            emit_x_dma(NCH + 2)
                        elif si == 6:
                            emit_x_dma(NCH + 3)
                flush_deferred()
                nc.gpsimd.collective_compute(
                    "AllToAll",
                    mybir.AluOpType.bypass,
                    replica_groups=[list(range(NCORES))],
                    ins=[outc_dr0.opt()],
                    outs=[at_dr0.opt()],
                )

                # ---- phase 2b: rest of proj(b1) under the b0 exchange ----
                for ci in (2, 3):
                    emit_proj_qk(1, ci)
                    emit_proj_v(1, ci)

                # x pool no longer needed: free it for the o-phase M tiles
                ctx_x.close()
                opool = ctx_x.enter_context(tc.tile_pool(name="opool", bufs=1))
                oev = ctx_x.enter_context(tc.tile_pool(name="oev", bufs=2))
                m_tiles = []
                for g in range(2):
                    for part, M_d in ((0, Mre_d), (1, Mim_d)):
                        m_sb = opool.tile(
                            [128, KT, 512], BF16, name=f"m{g}{part}", tag=f"m{g}{part}"
                        )
                        eng = nc.scalar if part == 0 else nc.sync
                        eng.dma_start(m_sb[:], M_d[:, g, :, :])
                        m_tiles.append((g, part, m_sb))
                bo_sb = [None, None]
                bo_sb[0] = keep.tile([128, 8], F32, name="bo_re", tag="bo_re")
                bo_sb[1] = keep.tile([128, 8], F32, name="bo_im", tag="bo_im")
                nc.gpsimd.dma_start(bo_sb[0][:], bore_d[:])
                nc.gpsimd.dma_start(bo_sb[1][:], boim_d[:])
                at_sb = [None, None]
                at_sb[0] = opool.tile(
                    [128, HPC, NCORES, TSL], BF16, name="at0", tag="at0"
                )
                at_sb[1] = opool.tile(
                    [128, HPC, NCORES, TSL], BF16, name="at1", tag="at1"
                )
                # at0 load on the gpsimd queue right after the A2A it waits on
                at0_t = at_dr0.rearrange("r (hp p) t -> p hp r t", p=128)
                for h in range(HPC):  # DMA APs are limited to 3 dims
                    nc.gpsimd.dma_start(at_sb[0][:, h, :, :], at0_t[:, h, :, :])

                def emit_oproj_group(b, g, part, m_sb):
                    """One quarter of the o-projection for batch b: 4 chains
                    of 128 output channels x TSL tokens + bias + store."""
                    otiles = [
                        pair_ps.tile([128, 1024], F32, name="prps", tag="prps"),
                        pair_ps.tile([128, 1024], F32, name="prps", tag="prps"),
                    ]
                    y_sb = oev.tile([128, 4, TSL], F32, name="y_sb", tag="y_sb")
                    for i in range(4):
                        # alternate tiles/half-banks: ACT reads chain i-1's
                        # bank while the PE accumulates into another
                        ps = otiles[i % 2][:, (i // 2) * 512 : (i // 2) * 512 + TSL]
                        for kt in range(KT):
                            # contraction row kt = (src rank kt//2, head kt%2)
                            nc.tensor.matmul(
                                ps,
                                m_sb[:, kt, i * 128 : (i + 1) * 128],
                                at_sb[b][:, kt % 2, kt // 2, :],
                                start=(kt == 0),
                                stop=(kt == KT - 1),
                            )
                        nc.scalar.activation(
                            y_sb[:, i, :],
                            ps,
                            mybir.ActivationFunctionType.Identity,
                            bias=bo_sb[part][:, g * 4 + i : g * 4 + i + 1],
                        )
                    cb0 = part * 8 + g * 4
                    nc.sync.dma_start(
                        yout_t[:, cb0 : cb0 + 4, b * TSL : (b + 1) * TSL], y_sb[:]
                    )

                # ---- phase 3: attention(b1) interleaved with o-proj(b0).
                # Groups go after mid-head qts only: a group after the last
                # qt of a head would delay that head's outc stores and A2A ----
                og = [(0, 2), (1, 0), (1, 2)]
                for hh in range(HPC):
                    for qt in range(N // 512):
                        emit_attn_qt(1, hh, qt)
                        if (hh, qt) in og:
                            gi = og.index((hh, qt))
                            emit_oproj_group(0, *m_tiles[gi][:2], m_tiles[gi][2])
                    flush_deferred()
                    # head hh's rows ship while the next head computes
                    nc.gpsimd.collective_compute(
                        "AllToAll",
                        mybir.AluOpType.bypass,
                        replica_groups=[list(range(NCORES))],
                        ins=[outc_dr1[hh].opt()],
                        outs=[at_dr1[hh].opt()],
                    )

                # ---- phase 4: rest of o-proj(b0) under the last A2A ----
                for gi in (3,):
                    emit_oproj_group(0, *m_tiles[gi][:2], m_tiles[gi][2])
                for h in range(HPC):
                    at1h_t = at_dr1[h].rearrange("r p t -> p r t")
                    nc.gpsimd.dma_start(at_sb[1][:, h, :, :], at1h_t[:, :, :])
                for gi in range(4):
                    emit_oproj_group(1, *m_tiles[gi][:2], m_tiles[gi][2])
                ctx_x.close()  # opool/oev close before the outer pools (LIFO)
    nc.compile()
    return nc


_NC_CACHE = None


def _get_program():
    global _NC_CACHE
    if _NC_CACHE is None:
        _NC_CACHE = _build_program()
    return _NC_CACHE


def _run(inputs, trace=False, trace_kwargs=None):
    shared, per_core = _host_prep(inputs)
    nc = _get_program()
    in_maps = []
    for c in range(NCORES):
        d = per_core[c]
        in_maps.append(
            {
                "xb": shared["xb"],
                "wq": d["wq"],
                "wk": d["wk"],
                "wv": d["wv"],
                "bq": d["bq"],
                "M_re": shared["M_re"],
                "M_im": shared["M_im"],
                "bo_re": shared["bo_re"],
                "bo_im": shared["bo_im"],
            }
        )
    res = run_bass_kernel_spmd(
        nc, in_maps, list(range(NCORES)), trace=trace, **(trace_kwargs or {})
    )
    youts = [res.results[c]["yout"] for c in range(NCORES)]
    # youts[c]: [2C, B*TSL]; rows [re(1024); im(1024)], cols [b0 256 | b1 256]
    re = np.zeros((B, N, C), dtype=np.float32)
    im = np.zeros((B, N, C), dtype=np.float32)
    for c in range(NCORES):
        for b in range(B):
            tsl = slice(c * TSL, (c + 1) * TSL)
            re[b, tsl] = youts[c][:C, b * TSL : (b + 1) * TSL].T
            im[b, tsl] = youts[c][C:, b * TSL : (b + 1) * TSL].T
    return np.stack([re, im]).astype(np.float32), res


def kernel(**inputs) -> np.ndarray:
    out, _ = _run(inputs, trace=False)
    return out


# revision 22
# speedup vs baseline: 1.0254x; 1.0067x over previous
"""Complex self-attention on 8 Trainium2 NeuronCores (Bass/Tile).

Model (reference): complex linear q/k/v projections of (x_re, x_im), attention
scores = (Re(q)·Re(k) + Im(q)·Im(k))/sqrt(D), softmax, attn applied to Re(v)
and Im(v), complex output projection. B=2, N=2048, C=1024, H=16, D=64.

Sharding: heads tensor-parallel across 8 cores (2 heads/core, both batches).
  - Projections: stacked-contraction trick ([x_re; x_im], 2C rows) against
    host-combined weights — each complex part is ONE bf16 matmul chain.
  - Software-pipelined phase schedule keeps the PE continuously busy (TRN2
    drops to half clock for 3us after ANY idle gap):
      phase 1: proj(b0)
      phase 2: attention(b0) interleaved with proj(b1)  [exp-bound slots
               filled with projection chains]
      phase 3: attention(b1) interleaved with half of o-proj(b0)
      phase 4: rest of o-proj(b0) under the last A2A, then o-proj(b1)
    The softmax tail of each query tile (den matmuls, reciprocal, scale,
    store) is deferred into the NEXT tile's emission so the PE never waits
    for the exp/DVE pipeline at tile boundaries.
  - Softmax denominator: exp tiles are tree-summed on the idle DVE in bf16,
    then 4 ones-matmuls per query tile (instead of 16) do the partition sum.
  - 1/den via reciprocal_approx_fast (~5x faster, 18-bit accurate).
  - k bias dropped (exact by softmax shift invariance); v bias folded into
    the o-projection bias on the host (softmax rows sum to 1).
  - A2A: batch 0 in one exchange (hidden under phase 3), batch 1 per head so
    only the last 0.5 MB exchange is exposed.
"""

import sys

if "/opt/trn_rl_repo" not in sys.path:
    sys.path.insert(0, "/opt/trn_rl_repo")

from contextlib import ExitStack

import ml_dtypes
import numpy as np

import concourse.mybir as mybir
import concourse.tile as tile
from concourse import bacc
from concourse.bass_utils import run_bass_kernel_spmd

B, N, C = 2, 2048, 1024
H, D = 16, 64
T = B * N  # 4096 tokens total
NCORES = 8
HPC = H // NCORES  # 2 heads per core
TSL = N // NCORES  # 256-token output slice per core PER BATCH
TF = 512  # projection token-chunk (free dim)
KT = 2 * C // 128  # 16 contraction tiles of 128 over [x_re; x_im]
NCH = N // TF  # token chunks per batch (4)
NKP = N // 256  # key-tile PAIRS per query tile (8): each pair = 2 x 128 keys
F32 = mybir.dt.float32
BF16 = mybir.dt.bfloat16
FP8 = mybir.dt.float8e4
BF = ml_dtypes.bfloat16

# fp8e4m3 exp/v with DoubleRow av+den measured FASTER on paper but fails the
# correctness gate (2.8e-2 rel err: weighted-sum quantization noise does not
# average down) and LDWEIGHTS-bound in practice. Keep the bf16 path.
USE_FP8_AV = False


def _host_prep(inp):
    """Build the host-side sharded/combined arrays (all matmul inputs bf16,
    blocked so every DMA is contiguous per partition)."""
    x_re = np.asarray(inp["x_re"], dtype=np.float32).reshape(T, C)
    x_im = np.asarray(inp["x_im"], dtype=np.float32).reshape(T, C)
    xT2 = np.concatenate([x_re.T, x_im.T], axis=0).astype(BF)  # [2C, T]
    # blocked: [p, chunk, kt, t] so a chunk load is 16KB contiguous/partition
    xb = np.ascontiguousarray(
        xT2.reshape(KT, 128, B * NCH, TF).transpose(1, 2, 0, 3)
    )

    per_core = []
    for c in range(NCORES):
        d = {}
        h0 = c * HPC
        ch = slice(h0 * D, (h0 + HPC) * D)
        for nm in ("q", "k", "v"):
            Wre = np.asarray(inp[f"{nm}_Wre"], dtype=np.float32)[ch]  # [128, C]
            Wim = np.asarray(inp[f"{nm}_Wim"], dtype=np.float32)[ch]
            bre = np.asarray(inp[f"{nm}_bre"], dtype=np.float32)[ch]
            bim = np.asarray(inp[f"{nm}_bim"], dtype=np.float32)[ch]
            Ws, bs = [], []
            for hh in range(HPC):
                hs = slice(hh * D, (hh + 1) * D)
                wr = np.concatenate([Wre[hs].T, -Wim[hs].T], axis=0)  # [2C, 64]
                wi = np.concatenate([Wim[hs].T, Wre[hs].T], axis=0)
                Ws.append(np.concatenate([wr, wi], axis=1))  # [2C, 128]
                bs.append(np.concatenate([bre[hs] - bim[hs], bre[hs] + bim[hs]]))
            if nm == "v":
                wvb = np.concatenate(Ws, axis=1).astype(BF)  # [2C, 256]
                d["wv"] = np.ascontiguousarray(
                    wvb.reshape(KT, 128, 2 * HPC * D).transpose(1, 0, 2)
                )  # [p, kt, 256]
            else:
                wqk = np.stack(Ws).astype(BF)  # [HPC, 2C, 128]
                d[f"w{nm}"] = np.ascontiguousarray(
                    wqk.reshape(HPC, KT, 128, 128).transpose(2, 0, 1, 3)
                )  # [p, hh, kt, m]
                if nm == "q":
                    d["bq"] = np.ascontiguousarray(np.stack(bs, axis=1))  # [128, HPC]
        per_core.append(d)

    # o-projection combined matrices, rows ordered to match the A2A result:
    # src rank r, then per rank [h0:out_r(64), h0:out_i(64), h1:..., h1:...]
    oWre = np.asarray(inp["o_Wre"], dtype=np.float32)
    oWim = np.asarray(inp["o_Wim"], dtype=np.float32)
    Mre_rows, Mim_rows = [], []
    bv_rows = []
    vbre = np.asarray(inp["v_bre"], dtype=np.float32)
    vbim = np.asarray(inp["v_bim"], dtype=np.float32)
    for r in range(NCORES):
        for hh in range(HPC):
            h = r * HPC + hh
            hs = slice(h * D, (h + 1) * D)
            Mre_rows += [oWre[:, hs].T, -oWim[:, hs].T]
            Mim_rows += [oWim[:, hs].T, oWre[:, hs].T]
            bv_rows += [vbre[hs] - vbim[hs], vbre[hs] + vbim[hs]]
    M_re = np.concatenate(Mre_rows, axis=0)  # [2C, C]
    M_im = np.concatenate(Mim_rows, axis=0)
    bv_full = np.concatenate(bv_rows)  # [2C] — v bias in A2A row order
    o_bre = np.asarray(inp["o_bre"], dtype=np.float32)
    o_bim = np.asarray(inp["o_bim"], dtype=np.float32)
    # fold the v bias through the o-projection (softmax rows sum to 1)
    bo_re = (o_bre - o_bim) + M_re.T @ bv_full  # [C]
    bo_im = (o_bre + o_bim) + M_im.T @ bv_full
    bo_re = np.ascontiguousarray(bo_re.reshape(8, 128).T.astype(np.float32))  # [128, 8]
    bo_im = np.ascontiguousarray(bo_im.reshape(8, 128).T.astype(np.float32))

    def mblk(M):  # [2C, C] -> [p, g, kt, 512] contiguous per partition
        Mb = M.astype(BF).reshape(KT, 128, 2, 512)
        return np.ascontiguousarray(Mb.transpose(1, 2, 0, 3))

    shared = dict(
        xb=xb, M_re=mblk(M_re), M_im=mblk(M_im), bo_re=bo_re, bo_im=bo_im
    )
    return shared, per_core


def _build_program():
    nc = bacc.Bacc("TRN2", target_bir_lowering=False, debug=False, num_devices=NCORES)

    # ---- DRAM I/O (host-blocked layouts: contiguous per partition) ----
    xb_d = nc.dram_tensor("xb", [128, B * NCH, KT, TF], BF16, kind="ExternalInput")
    wq_d = nc.dram_tensor("wq", [128, HPC, KT, 128], BF16, kind="ExternalInput")
    wk_d = nc.dram_tensor("wk", [128, HPC, KT, 128], BF16, kind="ExternalInput")
    wv_d = nc.dram_tensor("wv", [128, KT, 2 * HPC * D], BF16, kind="ExternalInput")
    bq_d = nc.dram_tensor("bq", [128, HPC], F32, kind="ExternalInput")
    Mre_d = nc.dram_tensor("M_re", [128, 2, KT, 512], BF16, kind="ExternalInput")
    Mim_d = nc.dram_tensor("M_im", [128, 2, KT, 512], BF16, kind="ExternalInput")
    bore_d = nc.dram_tensor("bo_re", [128, 8], F32, kind="ExternalInput")
    boim_d = nc.dram_tensor("bo_im", [128, 8], F32, kind="ExternalInput")
    # per-core output: rows = [re(1024); im(1024)], cols = [b0 slice | b1 slice]
    yout_d = nc.dram_tensor("yout", [2 * C, B * TSL], F32, kind="ExternalOutput")

    yout_t = yout_d.rearrange("(cb p) t -> p cb t", p=128)  # [128, 16, 512]

    with (
        tile.TileContext(nc) as tc,
        nc.allow_low_precision(
            reason="bf16 intermediates; rounding matches low-precision matmul noise"
        ),
    ):
        with tc.tile_pool(name="dram", bufs=1, space="DRAM") as dram:
            # A2A buffers: [dest rank, ch, TSL tok]; batch 0 whole, b1 per head
            outc_dr0 = dram.tile([NCORES, 256, TSL], BF16, name="outc0", tag="outc0")
            at_dr0 = dram.tile([NCORES, 256, TSL], BF16, name="at0d", tag="at0d")
            outc_dr1 = [
                dram.tile([NCORES, 128, TSL], BF16, name=f"outc1{h}", tag=f"outc1{h}")
                for h in range(HPC)
            ]
            at_dr1 = [
                dram.tile([NCORES, 128, TSL], BF16, name=f"at1{h}", tag=f"at1{h}")
                for h in range(HPC)
            ]

            with (
                tc.tile_pool(name="keep", bufs=1) as keep,
                # PSUM: pair(2 banks x2) + av(1 bank x2) + den(1 bank x2) = 8.
                tc.tile_pool(name="pair_ps", bufs=2, space="PSUM") as pair_ps,
                tc.tile_pool(name="av_ps", bufs=2, space="PSUM") as av_ps,
                tc.tile_pool(name="den_ps", bufs=2, space="PSUM") as den_ps,
                tc.tile_pool(name="qk_sb", bufs=1) as qk_sb,
                tc.tile_pool(name="v_sb", bufs=1) as v_sbp,
                tc.tile_pool(name="expp", bufs=4) as expp,
                tc.tile_pool(name="dtp", bufs=2) as dtp,
                tc.tile_pool(name="evp", bufs=3) as evp,
                tc.tile_pool(name="const", bufs=1) as const,
            ):
                ctx_x = ExitStack()
                xp = ctx_x.enter_context(tc.tile_pool(name="xp", bufs=2))

                # ---- startup: first x chunk split per-2kt across BOTH data
                # rings; weights per-kt on the gpsimd ring so the first chain
                # starts ~2us in ----
                xtiles = {}
                xt0 = xp.tile([128, KT, TF], BF16, name="xt", tag="xt")
                xtiles[0] = xt0
                for ks in range(0, KT, 2):
                    eng = nc.sync if (ks // 2) % 2 == 0 else nc.scalar
                    eng.dma_start(xt0[:, ks : ks + 2, :], xb_d[:, 0, ks : ks + 2, :])
                wq_sb = const.tile([128, HPC, KT, 128], BF16)
                wk_sb = const.tile([128, HPC, KT, 128], BF16)
                wv_sb = const.tile([128, KT, 2 * HPC * D], BF16)
                for kt in range(KT):
                    nc.gpsimd.dma_start(wq_sb[:, :, kt, :], wq_d[:, :, kt, :])
                nc.gpsimd.dma_start(wk_sb[:], wk_d[:])
                nc.gpsimd.dma_start(wv_sb[:], wv_d[:])
                bq_sb = keep.tile([128, HPC], F32)
                nc.gpsimd.dma_start(bq_sb[:], bq_d[:])
                if USE_FP8_AV:
                    # all-ones stationary for the DoubleRow denominator matmul
                    ones_t = keep.tile([128, 2, 128], FP8)
                else:
                    ones_t = keep.tile([128, 128], BF16)
                nc.vector.memset(ones_t[:], 1.0)
                # softmax shift (exact): score/8 tops out near +8.1, and
                # fp8e4m3 saturates at 448 — shift so max exp is ~e^4.7
                negone = keep.tile([128, 1], F32)
                nc.vector.memset(negone[:], -3.5)

                qc = {}
                kc = {}
                vt = {}
                for b in range(B):
                    for hh in range(HPC):
                        qc[b, hh] = qk_sb.tile(
                            [128, N], BF16, name=f"qc{b}{hh}", tag=f"qc{b}{hh}"
                        )
                        kc[b, hh] = qk_sb.tile(
                            [128, N], BF16, name=f"kc{b}{hh}", tag=f"kc{b}{hh}"
                        )
                        # fp8: [kp pair, 2, ch] layout feeds DoubleRow directly
                        vt[b, hh] = v_sbp.tile(
                            [128, N // 256, 2, 128] if USE_FP8_AV else [128, N // 128, 128],
                            FP8 if USE_FP8_AV else BF16,
                            name=f"vt{b}{hh}",
                            tag=f"vt{b}{hh}",
                        )

                def emit_x_dma(cg):
                    """Prefetch x chunk cg (global index)."""
                    xt = xp.tile([128, KT, TF], BF16, name="xt", tag="xt")
                    xtiles[cg] = xt
                    eng = nc.scalar if cg % 2 == 0 else nc.sync
                    eng.dma_start(xt[:], xb_d[:, cg, :, :])

                def emit_proj_qk(b, ci):
                    """q and k chains for chunk ci of batch b."""
                    cg = b * NCH + ci
                    xt = xtiles[cg]
                    csl = slice(ci * TF, ci * TF + TF)
                    prs = {}
                    for hh in range(HPC):
                        prs[hh] = pair_ps.tile([128, 1024], F32, name="prps", tag="prps")
                        for kt in range(KT):
                            nc.tensor.matmul(
                                prs[hh][:, 0:512],
                                wq_sb[:, hh, kt, :],
                                xt[:, kt, :],
                                start=(kt == 0),
                                stop=(kt == KT - 1),
                            )
                        nc.scalar.activation(
                            qc[b, hh][:, csl],
                            prs[hh][:, 0:512],
                            mybir.ActivationFunctionType.Identity,
                            bias=bq_sb[:, hh : hh + 1],
                        )
                    for hh in range(HPC):
                        for kt in range(KT):
                            nc.tensor.matmul(
                                prs[hh][:, 512:1024],
                                wk_sb[:, hh, kt, :],
                                xt[:, kt, :],
                                start=(kt == 0),
                                stop=(kt == KT - 1),
                            )
                        nc.vector.tensor_copy(kc[b, hh][:, csl], prs[hh][:, 512:1024])

                def emit_proj_v(b, ci):
                    """v chains for chunk ci of batch b."""
                    cg = b * NCH + ci
                    xt = xtiles[cg]
                    vtiles = [
                        pair_ps.tile([128, 1024], F32, name="prps", tag="prps")
                        for _ in range(2)
                    ]
                    for m in range(TF // 128):
                        # alternate tiles and half-banks so the DVE read of
                        # chain m-1 never shares a bank with chain m's write
                        vp = vtiles[m % 2][:, (m // 2) * 512 : (m // 2) * 512 + 256]
                        for kt in range(KT):
                            nc.tensor.matmul(
                                vp,
                                xt[:, kt, m * 128 : (m + 1) * 128],
                                wv_sb[:, kt, :],
                                start=(kt == 0),
                                stop=(kt == KT - 1),
                            )
                        ktok = ci * (TF // 128) + m
                        for hh in range(HPC):
                            dst = (
                                vt[b, hh][:, ktok // 2, ktok % 2, :]
                                if USE_FP8_AV
                                else vt[b, hh][:, ktok, :]
                            )
                            nc.vector.tensor_copy(
                                dst, vp[:, hh * 128 : (hh + 1) * 128]
                            )

                deferred = [None]  # single-slot pipeline for softmax tails

                def flush_deferred():
                    if deferred[0] is not None:
                        deferred[0]()
                        deferred[0] = None

                def emit_attn_qt(b, hh, qt):
                    """Scores/exp/av for one 512-query tile; the softmax tail
                    (den matmuls, reciprocal, scale, store) is deferred into
                    the next tile so the PE never idles at tile boundaries."""
                    qsl = slice(qt * 512, (qt + 1) * 512)
                    av = av_ps.tile([128, 512], F32, name="avps", tag="avps")

                    def _av(kp, ex):
                        for j in range(2):
                            kt = 2 * kp + j
                            nc.tensor.matmul(
                                av[:],
                                vt[b, hh][:, kt, :],
                                ex[:, j, :],
                                start=(kt == 0),
                                stop=(kt == N // 128 - 1),
                            )

                    pend = []
                    exs = []
                    pairs = {}
                    quads = {}
                    for kp in range(NKP):
                        pr = pair_ps.tile([128, 1024], F32, name="prps", tag="prps")
                        for j in range(2):
                            kt = 2 * kp + j
                            nc.tensor.matmul(
                                pr[:, j * 512 : (j + 1) * 512],
                                kc[b, hh][:, kt * 128 : (kt + 1) * 128],
                                qc[b, hh][:, qsl],
                                start=True,
                                stop=True,
                            )
                        ex = expp.tile([128, 2, 512], BF16, name="ex", tag="ex")
                        nc.scalar.activation(
                            ex[:, :, :],
                            pr[:],
                            mybir.ActivationFunctionType.Exp,
                            scale=0.125,
                        )
                        exs.append(ex)
                        # denominator tree-adds on the idle DVE (bf16 2x mode)
                        if kp % 2 == 1:
                            p = kp // 2
                            pairs[p] = dtp.tile(
                                [128, 1024], BF16, name=f"dp{p % 2}", tag=f"dp{p % 2}"
                            )
                            nc.vector.tensor_tensor(
                                pairs[p][:], exs[kp - 1][:], ex[:], mybir.AluOpType.add
                            )
                        if kp % 4 == 3:
                            q4 = kp // 4
                            quads[q4] = dtp.tile(
                                [128, 1024], BF16, name=f"dq{q4}", tag=f"dq{q4}"
                            )
                            nc.vector.tensor_tensor(
                                quads[q4][:],
                                pairs[kp // 2 - 1][:],
                                pairs[kp // 2][:],
                                mybir.AluOpType.add,
                            )
                        pend.append((kp, ex))
                        if kp == 1:
                            # PE is 2 key-pairs into this tile: safe point to
                            # emit the previous tile's softmax tail
                            flush_deferred()
                        if len(pend) > 2:
                            _av(*pend.pop(0))
                    pend_tail = list(pend)

                    def tail():
                        # last two av accumulations land here so the PE never
                        # waits on the exp pipeline at the tile boundary
                        for item in pend_tail:
                            _av(*item)
                        den = den_ps.tile([128, 512], F32, name="denps", tag="denps")
                        for i4, q4t in enumerate((quads[0], quads[1])):
                            for j in range(2):
                                nc.tensor.matmul(
                                    den[:],
                                    ones_t[:],
                                    q4t[:, j * 512 : (j + 1) * 512],
                                    start=(i4 == 0 and j == 0),
                                    stop=(i4 == 1 and j == 1),
                                )
                        rb = evp.tile([128, 512], F32, name="rb", tag="rb")
                        nc.vector.reciprocal_approx_fast(rb[:], den[:])
                        outc = evp.tile([128, 512], BF16, name="outc", tag="outc")
                        nc.vector.tensor_tensor(
                            outc[:], av[:], rb[:], mybir.AluOpType.mult
                        )
                        for j in range(2):
                            if b == 0:
                                dst = outc_dr0[2 * qt + j, hh * 128 : (hh + 1) * 128, :]
                            else:
                                dst = outc_dr1[hh][2 * qt + j, :, :]
                            nc.sync.dma_start(dst, outc[:, j * TSL : (j + 1) * TSL])

                    deferred[0] = tail

                # ---- phase 1: projections for batch 0 ----
                for ci in range(NCH):
                    if ci + 1 < NCH:
                        emit_x_dma(ci + 1)
                    emit_proj_qk(0, ci)
                    emit_proj_v(0, ci)
                emit_x_dma(NCH)  # first two b1 chunks prefetch here
                emit_x_dma(NCH + 1)

                # ---- phase 2a: attention(b0) with HALF of proj(b1) woven in
                # (fills the exp-bound slack and keeps the PE at full clock);
                # the other half runs after the b0 A2A fires, hiding the
                # first collective's cross-core skew sync (~17us) ----
                for hh in range(HPC):
                    for qt in range(N // 512):
                        emit_attn_qt(0, hh, qt)
                        si = hh * 4 + qt
                        if si == 0:
                            emit_proj_qk(1, 0)
                        elif si == 1:
                            emit_proj_v(1, 0)
                        elif si == 2:
                            emit_proj_qk(1, 1)
                        elif si == 3:
                            emit_proj_v(1, 1)
                        elif si == 4:
                            emit_x_dma(NCH + 2)
                        elif si == 6:
                            emit_x_dma(NCH + 3)
                flush_deferred()
                nc.gpsimd.collective_compute(
                    "AllToAll",
                    mybir.AluOpType.bypass,
                    replica_groups=[list(range(NCORES))],
                    ins=[outc_dr0.opt()],
                    outs=[at_dr0.opt()],
                )

                # ---- phase 2b: rest of proj(b1) under the b0 exchange ----
                for ci in (2, 3):
                    emit_proj_qk(1, ci)
                    emit_proj_v(1, ci)

                # x pool no longer needed: free it for the o-phase M tiles
                ctx_x.close()
                opool = ctx_x.enter_context(tc.tile_pool(name="opool", bufs=1))
                oev = ctx_x.enter_context(tc.tile_pool(name="oev", bufs=2))
                m_tiles = []
                for g in range(2):
                    for part, M_d in ((0, Mre_d), (1, Mim_d)):
                        m_sb = opool.tile(
                            [128, KT, 512], BF16, name=f"m{g}{part}", tag=f"m{g}{part}"
                        )
                        eng = nc.scalar if part == 0 else nc.sync
                        eng.dma_start(m_sb[:], M_d[:, g, :, :])
                        m_tiles.append((g, part, m_sb))
                bo_sb = [None, None]
                bo_sb[0] = keep.tile([128, 8], F32, name="bo_re", tag="bo_re")
                bo_sb[1] = keep.tile([128, 8], F32, name="bo_im", tag="bo_im")
                nc.gpsimd.dma_start(bo_sb[0][:], bore_d[:])
                nc.gpsimd.dma_start(bo_sb[1][:], boim_d[:])
                at_sb = [None, None]
                at_sb[0] = opool.tile(
                    [128, HPC, NCORES, TSL], BF16, name="at0", tag="at0"
                )
                at_sb[1] = opool.tile(
                    [128, HPC, NCORES, TSL], BF16, name="at1", tag="at1"
                )
                # at0 load on the gpsimd queue right after the A2A it waits on
                at0_t = at_dr0.rearrange("r (hp p) t -> p hp r t", p=128)
                for h in range(HPC):  # DMA APs are limited to 3 dims
                    nc.gpsimd.dma_start(at_sb[0][:, h, :, :], at0_t[:, h, :, :])

                def emit_oproj_group(b, g, part, m_sb):
                    """One quarter of the o-projection for batch b: 4 chains
                    of 128 output channels x TSL tokens + bias + store."""
                    otiles = [
                        pair_ps.tile([128, 1024], F32, name="prps", tag="prps"),
                        pair_ps.tile([128, 1024], F32, name="prps", tag="prps"),
                    ]
                    y_sb = oev.tile([128, 4, TSL], F32, name="y_sb", tag="y_sb")
                    for i in range(4):
                        # alternate tiles/half-banks: ACT reads chain i-1's
                        # bank while the PE accumulates into another
                        ps = otiles[i % 2][:, (i // 2) * 512 : (i // 2) * 512 + TSL]
                        for kt in range(KT):
                            # contraction row kt = (src rank kt//2, head kt%2)
                            nc.tensor.matmul(
                                ps,
                                m_sb[:, kt, i * 128 : (i + 1) * 128],
                                at_sb[b][:, kt % 2, kt // 2, :],
                                start=(kt == 0),
                                stop=(kt == KT - 1),
                            )
                        nc.scalar.activation(
                            y_sb[:, i, :],
                            ps,
                            mybir.ActivationFunctionType.Identity,
                            bias=bo_sb[part][:, g * 4 + i : g * 4 + i + 1],
                        )
                    cb0 = part * 8 + g * 4
                    nc.sync.dma_start(
                        yout_t[:, cb0 : cb0 + 4, b * TSL : (b + 1) * TSL], y_sb[:]
                    )

                # ---- phase 3: attention(b1) interleaved with o-proj(b0).
                # Groups go after mid-head qts only: a group after the last
                # qt of a head would delay that head's outc stores and A2A ----
                og = [(1, 0), (1, 1)]
                for hh in range(HPC):
                    for qt in range(N // 512):
                        emit_attn_qt(1, hh, qt)
                        if (hh, qt) in og:
                            gi = og.index((hh, qt))
                            emit_oproj_group(0, *m_tiles[gi][:2], m_tiles[gi][2])
                    flush_deferred()
                    # head hh's rows ship while the next head computes
                    nc.gpsimd.collective_compute(
                        "AllToAll",
                        mybir.AluOpType.bypass,
                        replica_groups=[list(range(NCORES))],
                        ins=[outc_dr1[hh].opt()],
                        outs=[at_dr1[hh].opt()],
                    )

                # ---- phase 4: rest of o-proj(b0) under the last A2A ----
                for gi in (2, 3):
                    emit_oproj_group(0, *m_tiles[gi][:2], m_tiles[gi][2])
                for h in range(HPC):
                    at1h_t = at_dr1[h].rearrange("r p t -> p r t")
                    nc.gpsimd.dma_start(at_sb[1][:, h, :, :], at1h_t[:, :, :])
                for gi in range(4):
                    emit_oproj_group(1, *m_tiles[gi][:2], m_tiles[gi][2])
                ctx_x.close()  # opool/oev close before the outer pools (LIFO)
    nc.compile()
    return nc


_NC_CACHE = None


def _get_program():
    global _NC_CACHE
    if _NC_CACHE is None:
        _NC_CACHE = _build_program()
    return _NC_CACHE


def _run(inputs, trace=False, trace_kwargs=None):
    shared, per_core = _host_prep(inputs)
    nc = _get_program()
    in_maps = []
    for c in range(NCORES):
        d = per_core[c]
        in_maps.append(
            {
                "xb": shared["xb"],
                "wq": d["wq"],
                "wk": d["wk"],
                "wv": d["wv"],
                "bq": d["bq"],
                "M_re": shared["M_re"],
                "M_im": shared["M_im"],
                "bo_re": shared["bo_re"],
                "bo_im": shared["bo_im"],
            }
        )
    res = run_bass_kernel_spmd(
        nc, in_maps, list(range(NCORES)), trace=trace, **(trace_kwargs or {})
    )
    youts = [res.results[c]["yout"] for c in range(NCORES)]
    # youts[c]: [2C, B*TSL]; rows [re(1024); im(1024)], cols [b0 256 | b1 256]
    re = np.zeros((B, N, C), dtype=np.float32)
    im = np.zeros((B, N, C), dtype=np.float32)
    for c in range(NCORES):
        for b in range(B):
            tsl = slice(c * TSL, (c + 1) * TSL)
            re[b, tsl] = youts[c][:C, b * TSL : (b + 1) * TSL].T
            im[b, tsl] = youts[c][C:, b * TSL : (b + 1) * TSL].T
    return np.stack([re, im]).astype(np.float32), res


def kernel(**inputs) -> np.ndarray:
    out, _ = _run(inputs, trace=False)
    return out


# revision 25
# speedup vs baseline: 1.0691x; 1.0426x over previous
"""Complex self-attention on 8 Trainium2 NeuronCores (Bass/Tile).

Model (reference): complex linear q/k/v projections of (x_re, x_im), attention
scores = (Re(q)·Re(k) + Im(q)·Im(k))/sqrt(D), softmax, attn applied to Re(v)
and Im(v), complex output projection. B=2, N=2048, C=1024, H=16, D=64.

Sharding: heads tensor-parallel across 8 cores (2 heads/core, both batches).
  - Projections: stacked-contraction trick ([x_re; x_im], 2C rows) against
    host-combined weights — each complex part is ONE bf16 matmul chain.
  - Software-pipelined phase schedule keeps the PE continuously busy (TRN2
    drops to half clock for 3us after ANY idle gap):
      phase 1: proj(b0)
      phase 2: attention(b0) interleaved with proj(b1)  [exp-bound slots
               filled with projection chains]
      phase 3: attention(b1) interleaved with half of o-proj(b0)
      phase 4: rest of o-proj(b0) under the last A2A, then o-proj(b1)
    The softmax tail of each query tile (den matmuls, reciprocal, scale,
    store) is deferred into the NEXT tile's emission so the PE never waits
    for the exp/DVE pipeline at tile boundaries.
  - Softmax denominator: exp tiles are tree-summed on the idle DVE in bf16,
    then 4 ones-matmuls per query tile (instead of 16) do the partition sum.
  - 1/den via reciprocal_approx_fast (~5x faster, 18-bit accurate).
  - k bias dropped (exact by softmax shift invariance); v bias folded into
    the o-projection bias on the host (softmax rows sum to 1).
  - A2A: batch 0 in one exchange (hidden under phase 3), batch 1 per head so
    only the last 0.5 MB exchange is exposed.
"""

import sys

if "/opt/trn_rl_repo" not in sys.path:
    sys.path.insert(0, "/opt/trn_rl_repo")

from contextlib import ExitStack

import ml_dtypes
import numpy as np

import concourse.mybir as mybir
import concourse.tile as tile
from concourse import bacc
from concourse.bass_utils import run_bass_kernel_spmd

B, N, C = 2, 2048, 1024
H, D = 16, 64
T = B * N  # 4096 tokens total
NCORES = 8
HPC = H // NCORES  # 2 heads per core
TSL = N // NCORES  # 256-token output slice per core PER BATCH
TF = 512  # projection token-chunk (free dim)
KT = 2 * C // 128  # 16 contraction tiles of 128 over [x_re; x_im]
NCH = N // TF  # token chunks per batch (4)
NKP = N // 256  # key-tile PAIRS per query tile (8): each pair = 2 x 128 keys
F32 = mybir.dt.float32
BF16 = mybir.dt.bfloat16
FP8 = mybir.dt.float8e4
BF = ml_dtypes.bfloat16

# fp8e4m3 exp/v with DoubleRow av+den measured FASTER on paper but fails the
# correctness gate (2.8e-2 rel err: weighted-sum quantization noise does not
# average down) and LDWEIGHTS-bound in practice. Keep the bf16 path.
USE_FP8_AV = False


def _host_prep(inp):
    """Build the host-side sharded/combined arrays (all matmul inputs bf16,
    blocked so every DMA is contiguous per partition)."""
    x_re = np.asarray(inp["x_re"], dtype=np.float32).reshape(T, C)
    x_im = np.asarray(inp["x_im"], dtype=np.float32).reshape(T, C)
    xT2 = np.concatenate([x_re.T, x_im.T], axis=0).astype(BF)  # [2C, T]
    # blocked: [p, chunk, kt, t] so a chunk load is 16KB contiguous/partition
    xb = np.ascontiguousarray(
        xT2.reshape(KT, 128, B * NCH, TF).transpose(1, 2, 0, 3)
    )

    per_core = []
    for c in range(NCORES):
        d = {}
        h0 = c * HPC
        ch = slice(h0 * D, (h0 + HPC) * D)
        for nm in ("q", "k", "v"):
            Wre = np.asarray(inp[f"{nm}_Wre"], dtype=np.float32)[ch]  # [128, C]
            Wim = np.asarray(inp[f"{nm}_Wim"], dtype=np.float32)[ch]
            bre = np.asarray(inp[f"{nm}_bre"], dtype=np.float32)[ch]
            bim = np.asarray(inp[f"{nm}_bim"], dtype=np.float32)[ch]
            Ws, bs = [], []
            for hh in range(HPC):
                hs = slice(hh * D, (hh + 1) * D)
                wr = np.concatenate([Wre[hs].T, -Wim[hs].T], axis=0)  # [2C, 64]
                wi = np.concatenate([Wim[hs].T, Wre[hs].T], axis=0)
                Ws.append(np.concatenate([wr, wi], axis=1))  # [2C, 128]
                bs.append(np.concatenate([bre[hs] - bim[hs], bre[hs] + bim[hs]]))
            if nm == "v":
                wvb = np.concatenate(Ws, axis=1).astype(BF)  # [2C, 256]
                d["wv"] = np.ascontiguousarray(
                    wvb.reshape(KT, 128, 2 * HPC * D).transpose(1, 0, 2)
                )  # [p, kt, 256]
            else:
                wqk = np.stack(Ws).astype(BF)  # [HPC, 2C, 128]
                d[f"w{nm}"] = np.ascontiguousarray(
                    wqk.reshape(HPC, KT, 128, 128).transpose(2, 0, 1, 3)
                )  # [p, hh, kt, m]
                if nm == "q":
                    d["bq"] = np.ascontiguousarray(np.stack(bs, axis=1))  # [128, HPC]
        per_core.append(d)

    # o-projection combined matrices, rows ordered to match the A2A result:
    # src rank r, then per rank [h0:out_r(64), h0:out_i(64), h1:..., h1:...]
    oWre = np.asarray(inp["o_Wre"], dtype=np.float32)
    oWim = np.asarray(inp["o_Wim"], dtype=np.float32)
    Mre_rows, Mim_rows = [], []
    bv_rows = []
    vbre = np.asarray(inp["v_bre"], dtype=np.float32)
    vbim = np.asarray(inp["v_bim"], dtype=np.float32)
    for r in range(NCORES):
        for hh in range(HPC):
            h = r * HPC + hh
            hs = slice(h * D, (h + 1) * D)
            Mre_rows += [oWre[:, hs].T, -oWim[:, hs].T]
            Mim_rows += [oWim[:, hs].T, oWre[:, hs].T]
            bv_rows += [vbre[hs] - vbim[hs], vbre[hs] + vbim[hs]]
    M_re = np.concatenate(Mre_rows, axis=0)  # [2C, C]
    M_im = np.concatenate(Mim_rows, axis=0)
    bv_full = np.concatenate(bv_rows)  # [2C] — v bias in A2A row order
    o_bre = np.asarray(inp["o_bre"], dtype=np.float32)
    o_bim = np.asarray(inp["o_bim"], dtype=np.float32)
    # fold the v bias through the o-projection (softmax rows sum to 1)
    bo_re = (o_bre - o_bim) + M_re.T @ bv_full  # [C]
    bo_im = (o_bre + o_bim) + M_im.T @ bv_full
    bo_re = np.ascontiguousarray(bo_re.reshape(8, 128).T.astype(np.float32))  # [128, 8]
    bo_im = np.ascontiguousarray(bo_im.reshape(8, 128).T.astype(np.float32))

    def mblk(M):  # [2C, C] -> [p, g, kt, 512] contiguous per partition
        Mb = M.astype(BF).reshape(KT, 128, 2, 512)
        return np.ascontiguousarray(Mb.transpose(1, 2, 0, 3))

    shared = dict(
        xb=xb, M_re=mblk(M_re), M_im=mblk(M_im), bo_re=bo_re, bo_im=bo_im
    )
    return shared, per_core


def _build_program():
    nc = bacc.Bacc("TRN2", target_bir_lowering=False, debug=False, num_devices=NCORES)

    # ---- DRAM I/O (host-blocked layouts: contiguous per partition) ----
    xb_d = nc.dram_tensor("xb", [128, B * NCH, KT, TF], BF16, kind="ExternalInput")
    wq_d = nc.dram_tensor("wq", [128, HPC, KT, 128], BF16, kind="ExternalInput")
    wk_d = nc.dram_tensor("wk", [128, HPC, KT, 128], BF16, kind="ExternalInput")
    wv_d = nc.dram_tensor("wv", [128, KT, 2 * HPC * D], BF16, kind="ExternalInput")
    bq_d = nc.dram_tensor("bq", [128, HPC], F32, kind="ExternalInput")
    Mre_d = nc.dram_tensor("M_re", [128, 2, KT, 512], BF16, kind="ExternalInput")
    Mim_d = nc.dram_tensor("M_im", [128, 2, KT, 512], BF16, kind="ExternalInput")
    bore_d = nc.dram_tensor("bo_re", [128, 8], F32, kind="ExternalInput")
    boim_d = nc.dram_tensor("bo_im", [128, 8], F32, kind="ExternalInput")
    # per-core output: rows = [re(1024); im(1024)], cols = [b0 slice | b1 slice]
    yout_d = nc.dram_tensor("yout", [2 * C, B * TSL], F32, kind="ExternalOutput")

    yout_t = yout_d.rearrange("(cb p) t -> p cb t", p=128)  # [128, 16, 512]

    with (
        tile.TileContext(nc) as tc,
        nc.allow_low_precision(
            reason="bf16 intermediates; rounding matches low-precision matmul noise"
        ),
    ):
        with tc.tile_pool(name="dram", bufs=1, space="DRAM") as dram:
            # A2A buffers: [dest rank, ch, TSL tok]; batch 0 whole, b1 per head
            outc_dr0 = dram.tile([NCORES, 256, TSL], BF16, name="outc0", tag="outc0")
            at_dr0 = dram.tile([NCORES, 256, TSL], BF16, name="at0d", tag="at0d")
            outc_dr1 = [
                dram.tile([NCORES, 128, TSL], BF16, name=f"outc1{h}", tag=f"outc1{h}")
                for h in range(HPC)
            ]
            at_dr1 = [
                dram.tile([NCORES, 128, TSL], BF16, name=f"at1{h}", tag=f"at1{h}")
                for h in range(HPC)
            ]

            with (
                tc.tile_pool(name="keep", bufs=1) as keep,
                # PSUM: pair(2 banks x2) + av(1 bank x2) + den(1 bank x2) = 8.
                tc.tile_pool(name="pair_ps", bufs=2, space="PSUM") as pair_ps,
                tc.tile_pool(name="av_ps", bufs=2, space="PSUM") as av_ps,
                tc.tile_pool(name="den_ps", bufs=2, space="PSUM") as den_ps,
                tc.tile_pool(name="qk_sb", bufs=1) as qk_sb,
                tc.tile_pool(name="v_sb", bufs=1) as v_sbp,
                tc.tile_pool(name="expp", bufs=4) as expp,
                tc.tile_pool(name="dtp", bufs=2) as dtp,
                tc.tile_pool(name="evp", bufs=3) as evp,
                tc.tile_pool(name="const", bufs=1) as const,
            ):
                ctx_x = ExitStack()
                xp = ctx_x.enter_context(tc.tile_pool(name="xp", bufs=2))

                # ---- startup: first x chunk split per-2kt across BOTH data
                # rings; weights per-kt on the gpsimd ring so the first chain
                # starts ~2us in ----
                xtiles = {}
                xt0 = xp.tile([128, KT, TF], BF16, name="xt", tag="xt")
                xtiles[0] = xt0
                for ks in range(0, KT, 2):
                    eng = nc.sync if (ks // 2) % 2 == 0 else nc.scalar
                    eng.dma_start(xt0[:, ks : ks + 2, :], xb_d[:, 0, ks : ks + 2, :])
                wq_sb = const.tile([128, HPC, KT, 128], BF16)
                wk_sb = const.tile([128, HPC, KT, 128], BF16)
                wv_sb = const.tile([128, KT, 2 * HPC * D], BF16)
                for kt in range(KT):
                    nc.scalar.dma_start(wq_sb[:, :, kt, :], wq_d[:, :, kt, :])
                nc.scalar.dma_start(wk_sb[:], wk_d[:])
                nc.gpsimd.dma_start(wv_sb[:], wv_d[:])
                bq_sb = keep.tile([128, HPC], F32)
                nc.gpsimd.dma_start(bq_sb[:], bq_d[:])
                if USE_FP8_AV:
                    # all-ones stationary for the DoubleRow denominator matmul
                    ones_t = keep.tile([128, 2, 128], FP8)
                else:
                    ones_t = keep.tile([128, 128], BF16)
                nc.vector.memset(ones_t[:], 1.0)
                # softmax shift (exact): score/8 tops out near +8.1, and
                # fp8e4m3 saturates at 448 — shift so max exp is ~e^4.7
                negone = keep.tile([128, 1], F32)
                nc.vector.memset(negone[:], -3.5)

                qc = {}
                kc = {}
                vt = {}
                for b in range(B):
                    for hh in range(HPC):
                        qc[b, hh] = qk_sb.tile(
                            [128, N], BF16, name=f"qc{b}{hh}", tag=f"qc{b}{hh}"
                        )
                        kc[b, hh] = qk_sb.tile(
                            [128, N], BF16, name=f"kc{b}{hh}", tag=f"kc{b}{hh}"
                        )
                        # fp8: [kp pair, 2, ch] layout feeds DoubleRow directly
                        vt[b, hh] = v_sbp.tile(
                            [128, N // 256, 2, 128] if USE_FP8_AV else [128, N // 128, 128],
                            FP8 if USE_FP8_AV else BF16,
                            name=f"vt{b}{hh}",
                            tag=f"vt{b}{hh}",
                        )

                def emit_x_dma(cg):
                    """Prefetch x chunk cg (global index)."""
                    xt = xp.tile([128, KT, TF], BF16, name="xt", tag="xt")
                    xtiles[cg] = xt
                    eng = nc.scalar if cg % 2 == 0 else nc.sync
                    eng.dma_start(xt[:], xb_d[:, cg, :, :])

                def emit_proj_qk(b, ci):
                    """q and k chains for chunk ci of batch b."""
                    cg = b * NCH + ci
                    xt = xtiles[cg]
                    csl = slice(ci * TF, ci * TF + TF)
                    prs = {}
                    for hh in range(HPC):
                        prs[hh] = pair_ps.tile([128, 1024], F32, name="prps", tag="prps")
                        for kt in range(KT):
                            nc.tensor.matmul(
                                prs[hh][:, 0:512],
                                wq_sb[:, hh, kt, :],
                                xt[:, kt, :],
                                start=(kt == 0),
                                stop=(kt == KT - 1),
                            )
                        nc.scalar.activation(
                            qc[b, hh][:, csl],
                            prs[hh][:, 0:512],
                            mybir.ActivationFunctionType.Identity,
                            bias=bq_sb[:, hh : hh + 1],
                        )
                    for hh in range(HPC):
                        for kt in range(KT):
                            nc.tensor.matmul(
                                prs[hh][:, 512:1024],
                                wk_sb[:, hh, kt, :],
                                xt[:, kt, :],
                                start=(kt == 0),
                                stop=(kt == KT - 1),
                            )
                        nc.vector.tensor_copy(kc[b, hh][:, csl], prs[hh][:, 512:1024])

                def emit_proj_v(b, ci):
                    """v chains for chunk ci of batch b."""
                    cg = b * NCH + ci
                    xt = xtiles[cg]
                    vtiles = [
                        pair_ps.tile([128, 1024], F32, name="prps", tag="prps")
                        for _ in range(2)
                    ]
                    for m in range(TF // 128):
                        # alternate tiles and half-banks so the DVE read of
                        # chain m-1 never shares a bank with chain m's write
                        vp = vtiles[m % 2][:, (m // 2) * 512 : (m // 2) * 512 + 256]
                        for kt in range(KT):
                            nc.tensor.matmul(
                                vp,
                                xt[:, kt, m * 128 : (m + 1) * 128],
                                wv_sb[:, kt, :],
                                start=(kt == 0),
                                stop=(kt == KT - 1),
                            )
                        ktok = ci * (TF // 128) + m
                        for hh in range(HPC):
                            dst = (
                                vt[b, hh][:, ktok // 2, ktok % 2, :]
                                if USE_FP8_AV
                                else vt[b, hh][:, ktok, :]
                            )
                            nc.vector.tensor_copy(
                                dst, vp[:, hh * 128 : (hh + 1) * 128]
                            )

                deferred = [None]  # single-slot pipeline for softmax tails

                def flush_deferred():
                    if deferred[0] is not None:
                        deferred[0]()
                        deferred[0] = None

                def emit_attn_qt(b, hh, qt):
                    """Scores/exp/av for one 512-query tile; the softmax tail
                    (den matmuls, reciprocal, scale, store) is deferred into
                    the next tile so the PE never idles at tile boundaries."""
                    qsl = slice(qt * 512, (qt + 1) * 512)
                    av = av_ps.tile([128, 512], F32, name="avps", tag="avps")

                    def _av(kp, ex):
                        for j in range(2):
                            kt = 2 * kp + j
                            nc.tensor.matmul(
                                av[:],
                                vt[b, hh][:, kt, :],
                                ex[:, j, :],
                                start=(kt == 0),
                                stop=(kt == N // 128 - 1),
                            )

                    pend = []
                    exs = []
                    pairs = {}
                    quads = {}
                    for kp in range(NKP):
                        pr = pair_ps.tile([128, 1024], F32, name="prps", tag="prps")
                        for j in range(2):
                            kt = 2 * kp + j
                            nc.tensor.matmul(
                                pr[:, j * 512 : (j + 1) * 512],
                                kc[b, hh][:, kt * 128 : (kt + 1) * 128],
                                qc[b, hh][:, qsl],
                                start=True,
                                stop=True,
                            )
                        ex = expp.tile([128, 2, 512], BF16, name="ex", tag="ex")
                        nc.scalar.activation(
                            ex[:, :, :],
                            pr[:],
                            mybir.ActivationFunctionType.Exp,
                            scale=0.125,
                        )
                        exs.append(ex)
                        # denominator tree-adds on the idle DVE (bf16 2x mode)
                        if kp % 2 == 1:
                            p = kp // 2
                            pairs[p] = dtp.tile(
                                [128, 1024], BF16, name=f"dp{p % 2}", tag=f"dp{p % 2}"
                            )
                            nc.vector.tensor_tensor(
                                pairs[p][:], exs[kp - 1][:], ex[:], mybir.AluOpType.add
                            )
                        if kp % 4 == 3:
                            q4 = kp // 4
                            quads[q4] = dtp.tile(
                                [128, 1024], BF16, name=f"dq{q4}", tag=f"dq{q4}"
                            )
                            nc.vector.tensor_tensor(
                                quads[q4][:],
                                pairs[kp // 2 - 1][:],
                                pairs[kp // 2][:],
                                mybir.AluOpType.add,
                            )
                        pend.append((kp, ex))
                        if kp == 1:
                            # PE is 2 key-pairs into this tile: safe point to
                            # emit the previous tile's softmax tail
                            flush_deferred()
                        if len(pend) > 2:
                            _av(*pend.pop(0))
                    pend_tail = list(pend)

                    def tail():
                        # last two av accumulations land here so the PE never
                        # waits on the exp pipeline at the tile boundary
                        for item in pend_tail:
                            _av(*item)
                        den = den_ps.tile([128, 512], F32, name="denps", tag="denps")
                        for i4, q4t in enumerate((quads[0], quads[1])):
                            for j in range(2):
                                nc.tensor.matmul(
                                    den[:],
                                    ones_t[:],
                                    q4t[:, j * 512 : (j + 1) * 512],
                                    start=(i4 == 0 and j == 0),
                                    stop=(i4 == 1 and j == 1),
                                )
                        rb = evp.tile([128, 512], F32, name="rb", tag="rb")
                        nc.vector.reciprocal_approx_fast(rb[:], den[:])
                        outc = evp.tile([128, 512], BF16, name="outc", tag="outc")
                        nc.vector.tensor_tensor(
                            outc[:], av[:], rb[:], mybir.AluOpType.mult
                        )
                        for j in range(2):
                            if b == 0:
                                dst = outc_dr0[2 * qt + j, hh * 128 : (hh + 1) * 128, :]
                            else:
                                dst = outc_dr1[hh][2 * qt + j, :, :]
                            nc.sync.dma_start(dst, outc[:, j * TSL : (j + 1) * TSL])

                    deferred[0] = tail

                # ---- phase 1: projections for batch 0 ----
                for ci in range(NCH):
                    if ci + 1 < NCH:
                        emit_x_dma(ci + 1)
                    emit_proj_qk(0, ci)
                    emit_proj_v(0, ci)
                emit_x_dma(NCH)  # first two b1 chunks prefetch here
                emit_x_dma(NCH + 1)

                # ---- phase 2a: attention(b0) with HALF of proj(b1) woven in
                # (fills the exp-bound slack and keeps the PE at full clock);
                # the other half runs after the b0 A2A fires, hiding the
                # first collective's cross-core skew sync (~17us) ----
                for hh in range(HPC):
                    for qt in range(N // 512):
                        emit_attn_qt(0, hh, qt)
                        si = hh * 4 + qt
                        if si == 0:
                            emit_proj_qk(1, 0)
                        elif si == 1:
                            emit_proj_v(1, 0)
                        elif si == 2:
                            emit_proj_qk(1, 1)
                        elif si == 3:
                            emit_proj_v(1, 1)
                        elif si == 4:
                            emit_x_dma(NCH + 2)
                        elif si == 6:
                            emit_x_dma(NCH + 3)
                flush_deferred()
                nc.gpsimd.collective_compute(
                    "AllToAll",
                    mybir.AluOpType.bypass,
                    replica_groups=[list(range(NCORES))],
                    ins=[outc_dr0.opt()],
                    outs=[at_dr0.opt()],
                )

                # ---- phase 2b: rest of proj(b1) under the b0 exchange ----
                for ci in (2, 3):
                    emit_proj_qk(1, ci)
                    emit_proj_v(1, ci)

                # x pool no longer needed: free it for the o-phase M tiles
                ctx_x.close()
                opool = ctx_x.enter_context(tc.tile_pool(name="opool", bufs=1))
                oev = ctx_x.enter_context(tc.tile_pool(name="oev", bufs=2))
                m_tiles = []
                for g in range(2):
                    for part, M_d in ((0, Mre_d), (1, Mim_d)):
                        m_sb = opool.tile(
                            [128, KT, 512], BF16, name=f"m{g}{part}", tag=f"m{g}{part}"
                        )
                        eng = nc.scalar if part == 0 else nc.sync
                        eng.dma_start(m_sb[:], M_d[:, g, :, :])
                        m_tiles.append((g, part, m_sb))
                bo_sb = [None, None]
                bo_sb[0] = keep.tile([128, 8], F32, name="bo_re", tag="bo_re")
                bo_sb[1] = keep.tile([128, 8], F32, name="bo_im", tag="bo_im")
                nc.gpsimd.dma_start(bo_sb[0][:], bore_d[:])
                nc.gpsimd.dma_start(bo_sb[1][:], boim_d[:])
                at_sb = [None, None]
                at_sb[0] = opool.tile(
                    [128, HPC, NCORES, TSL], BF16, name="at0", tag="at0"
                )
                at_sb[1] = opool.tile(
                    [128, HPC, NCORES, TSL], BF16, name="at1", tag="at1"
                )
                # at0 load on the gpsimd queue right after the A2A it waits on
                at0_t = at_dr0.rearrange("r (hp p) t -> p hp r t", p=128)
                for h in range(HPC):  # DMA APs are limited to 3 dims
                    nc.gpsimd.dma_start(at_sb[0][:, h, :, :], at0_t[:, h, :, :])

                def emit_oproj_group(b, g, part, m_sb):
                    """One quarter of the o-projection for batch b: 4 chains
                    of 128 output channels x TSL tokens + bias + store."""
                    otiles = [
                        pair_ps.tile([128, 1024], F32, name="prps", tag="prps"),
                        pair_ps.tile([128, 1024], F32, name="prps", tag="prps"),
                    ]
                    y_sb = oev.tile([128, 4, TSL], F32, name="y_sb", tag="y_sb")
                    for i in range(4):
                        # alternate tiles/half-banks: ACT reads chain i-1's
                        # bank while the PE accumulates into another
                        ps = otiles[i % 2][:, (i // 2) * 512 : (i // 2) * 512 + TSL]
                        for kt in range(KT):
                            # contraction row kt = (src rank kt//2, head kt%2)
                            nc.tensor.matmul(
                                ps,
                                m_sb[:, kt, i * 128 : (i + 1) * 128],
                                at_sb[b][:, kt % 2, kt // 2, :],
                                start=(kt == 0),
                                stop=(kt == KT - 1),
                            )
                        nc.scalar.activation(
                            y_sb[:, i, :],
                            ps,
                            mybir.ActivationFunctionType.Identity,
                            bias=bo_sb[part][:, g * 4 + i : g * 4 + i + 1],
                        )
                    cb0 = part * 8 + g * 4
                    nc.sync.dma_start(
                        yout_t[:, cb0 : cb0 + 4, b * TSL : (b + 1) * TSL], y_sb[:]
                    )

                # ---- phase 3: attention(b1) interleaved with o-proj(b0).
                # Groups go after mid-head qts only: a group after the last
                # qt of a head would delay that head's outc stores and A2A ----
                og = []
                for hh in range(HPC):
                    for qt in range(N // 512):
                        emit_attn_qt(1, hh, qt)
                        if (hh, qt) in og:
                            gi = og.index((hh, qt))
                            emit_oproj_group(0, *m_tiles[gi][:2], m_tiles[gi][2])
                    flush_deferred()
                    # head hh's rows ship while the next head computes
                    nc.gpsimd.collective_compute(
                        "AllToAll",
                        mybir.AluOpType.bypass,
                        replica_groups=[list(range(NCORES))],
                        ins=[outc_dr1[hh].opt()],
                        outs=[at_dr1[hh].opt()],
                    )

                # ---- phase 4: o-proj(b0) under the last A2A ----
                for gi in (0, 1, 2, 3):
                    emit_oproj_group(0, *m_tiles[gi][:2], m_tiles[gi][2])
                for h in range(HPC):
                    at1h_t = at_dr1[h].rearrange("r p t -> p r t")
                    nc.gpsimd.dma_start(at_sb[1][:, h, :, :], at1h_t[:, :, :])
                for gi in range(4):
                    emit_oproj_group(1, *m_tiles[gi][:2], m_tiles[gi][2])
                ctx_x.close()  # opool/oev close before the outer pools (LIFO)
    nc.compile()
    return nc


_NC_CACHE = None


def _get_program():
    global _NC_CACHE
    if _NC_CACHE is None:
        _NC_CACHE = _build_program()
    return _NC_CACHE


def _run(inputs, trace=False, trace_kwargs=None):
    shared, per_core = _host_prep(inputs)
    nc = _get_program()
    in_maps = []
    for c in range(NCORES):
        d = per_core[c]
        in_maps.append(
            {
                "xb": shared["xb"],
                "wq": d["wq"],
                "wk": d["wk"],
                "wv": d["wv"],
                "bq": d["bq"],
                "M_re": shared["M_re"],
                "M_im": shared["M_im"],
                "bo_re": shared["bo_re"],
                "bo_im": shared["bo_im"],
            }
        )
    res = run_bass_kernel_spmd(
        nc, in_maps, list(range(NCORES)), trace=trace, **(trace_kwargs or {})
    )
    youts = [res.results[c]["yout"] for c in range(NCORES)]
    # youts[c]: [2C, B*TSL]; rows [re(1024); im(1024)], cols [b0 256 | b1 256]
    re = np.zeros((B, N, C), dtype=np.float32)
    im = np.zeros((B, N, C), dtype=np.float32)
    for c in range(NCORES):
        for b in range(B):
            tsl = slice(c * TSL, (c + 1) * TSL)
            re[b, tsl] = youts[c][:C, b * TSL : (b + 1) * TSL].T
            im[b, tsl] = youts[c][C:, b * TSL : (b + 1) * TSL].T
    return np.stack([re, im]).astype(np.float32), res


def kernel(**inputs) -> np.ndarray:
    out, _ = _run(inputs, trace=False)
    return out


# revision 28
# speedup vs baseline: 1.0708x; 1.0016x over previous
"""Complex self-attention on 8 Trainium2 NeuronCores (Bass/Tile).

Model (reference): complex linear q/k/v projections of (x_re, x_im), attention
scores = (Re(q)·Re(k) + Im(q)·Im(k))/sqrt(D), softmax, attn applied to Re(v)
and Im(v), complex output projection. B=2, N=2048, C=1024, H=16, D=64.

Sharding: heads tensor-parallel across 8 cores (2 heads/core, both batches).
  - Projections: stacked-contraction trick ([x_re; x_im], 2C rows) against
    host-combined weights — each complex part is ONE bf16 matmul chain.
  - Software-pipelined phase schedule keeps the PE continuously busy (TRN2
    drops to half clock for 3us after ANY idle gap):
      phase 1: proj(b0)
      phase 2: attention(b0) interleaved with proj(b1)  [exp-bound slots
               filled with projection chains]
      phase 3: attention(b1) interleaved with half of o-proj(b0)
      phase 4: rest of o-proj(b0) under the last A2A, then o-proj(b1)
    The softmax tail of each query tile (den matmuls, reciprocal, scale,
    store) is deferred into the NEXT tile's emission so the PE never waits
    for the exp/DVE pipeline at tile boundaries.
  - Softmax denominator: exp tiles are tree-summed on the idle DVE in bf16,
    then 4 ones-matmuls per query tile (instead of 16) do the partition sum.
  - 1/den via reciprocal_approx_fast (~5x faster, 18-bit accurate).
  - k bias dropped (exact by softmax shift invariance); v bias folded into
    the o-projection bias on the host (softmax rows sum to 1).
  - A2A: batch 0 in one exchange (hidden under phase 3), batch 1 per head so
    only the last 0.5 MB exchange is exposed.
"""

import sys

if "/opt/trn_rl_repo" not in sys.path:
    sys.path.insert(0, "/opt/trn_rl_repo")

from contextlib import ExitStack

import ml_dtypes
import numpy as np

import concourse.mybir as mybir
import concourse.tile as tile
from concourse import bacc
from concourse.bass_utils import run_bass_kernel_spmd

B, N, C = 2, 2048, 1024
H, D = 16, 64
T = B * N  # 4096 tokens total
NCORES = 8
HPC = H // NCORES  # 2 heads per core
TSL = N // NCORES  # 256-token output slice per core PER BATCH
TF = 512  # projection token-chunk (free dim)
KT = 2 * C // 128  # 16 contraction tiles of 128 over [x_re; x_im]
NCH = N // TF  # token chunks per batch (4)
NKP = N // 256  # key-tile PAIRS per query tile (8): each pair = 2 x 128 keys
F32 = mybir.dt.float32
BF16 = mybir.dt.bfloat16
FP8 = mybir.dt.float8e4
BF = ml_dtypes.bfloat16

# fp8e4m3 exp/v with DoubleRow av+den measured FASTER on paper but fails the
# correctness gate (2.8e-2 rel err: weighted-sum quantization noise does not
# average down) and LDWEIGHTS-bound in practice. Keep the bf16 path.
USE_FP8_AV = False


def _host_prep(inp):
    """Build the host-side sharded/combined arrays (all matmul inputs bf16,
    blocked so every DMA is contiguous per partition)."""
    x_re = np.asarray(inp["x_re"], dtype=np.float32).reshape(T, C)
    x_im = np.asarray(inp["x_im"], dtype=np.float32).reshape(T, C)
    xT2 = np.concatenate([x_re.T, x_im.T], axis=0).astype(BF)  # [2C, T]
    # blocked: [p, chunk, kt, t] so a chunk load is 16KB contiguous/partition
    xb = np.ascontiguousarray(
        xT2.reshape(KT, 128, B * NCH, TF).transpose(1, 2, 0, 3)
    )

    per_core = []
    for c in range(NCORES):
        d = {}
        h0 = c * HPC
        ch = slice(h0 * D, (h0 + HPC) * D)
        for nm in ("q", "k", "v"):
            Wre = np.asarray(inp[f"{nm}_Wre"], dtype=np.float32)[ch]  # [128, C]
            Wim = np.asarray(inp[f"{nm}_Wim"], dtype=np.float32)[ch]
            bre = np.asarray(inp[f"{nm}_bre"], dtype=np.float32)[ch]
            bim = np.asarray(inp[f"{nm}_bim"], dtype=np.float32)[ch]
            Ws, bs = [], []
            for hh in range(HPC):
                hs = slice(hh * D, (hh + 1) * D)
                wr = np.concatenate([Wre[hs].T, -Wim[hs].T], axis=0)  # [2C, 64]
                wi = np.concatenate([Wim[hs].T, Wre[hs].T], axis=0)
                Ws.append(np.concatenate([wr, wi], axis=1))  # [2C, 128]
                bs.append(np.concatenate([bre[hs] - bim[hs], bre[hs] + bim[hs]]))
            if nm == "v":
                wvb = np.concatenate(Ws, axis=1).astype(BF)  # [2C, 256]
                d["wv"] = np.ascontiguousarray(
                    wvb.reshape(KT, 128, 2 * HPC * D).transpose(1, 0, 2)
                )  # [p, kt, 256]
            else:
                wqk = np.stack(Ws).astype(BF)  # [HPC, 2C, 128]
                d[f"w{nm}"] = np.ascontiguousarray(
                    wqk.reshape(HPC, KT, 128, 128).transpose(2, 0, 1, 3)
                )  # [p, hh, kt, m]
                if nm == "q":
                    d["bq"] = np.ascontiguousarray(np.stack(bs, axis=1))  # [128, HPC]
        per_core.append(d)

    # o-projection combined matrices, rows ordered to match the A2A result:
    # src rank r, then per rank [h0:out_r(64), h0:out_i(64), h1:..., h1:...]
    oWre = np.asarray(inp["o_Wre"], dtype=np.float32)
    oWim = np.asarray(inp["o_Wim"], dtype=np.float32)
    Mre_rows, Mim_rows = [], []
    bv_rows = []
    vbre = np.asarray(inp["v_bre"], dtype=np.float32)
    vbim = np.asarray(inp["v_bim"], dtype=np.float32)
    for r in range(NCORES):
        for hh in range(HPC):
            h = r * HPC + hh
            hs = slice(h * D, (h + 1) * D)
            Mre_rows += [oWre[:, hs].T, -oWim[:, hs].T]
            Mim_rows += [oWim[:, hs].T, oWre[:, hs].T]
            bv_rows += [vbre[hs] - vbim[hs], vbre[hs] + vbim[hs]]
    M_re = np.concatenate(Mre_rows, axis=0)  # [2C, C]
    M_im = np.concatenate(Mim_rows, axis=0)
    bv_full = np.concatenate(bv_rows)  # [2C] — v bias in A2A row order
    o_bre = np.asarray(inp["o_bre"], dtype=np.float32)
    o_bim = np.asarray(inp["o_bim"], dtype=np.float32)
    # fold the v bias through the o-projection (softmax rows sum to 1)
    bo_re = (o_bre - o_bim) + M_re.T @ bv_full  # [C]
    bo_im = (o_bre + o_bim) + M_im.T @ bv_full
    bo_re = np.ascontiguousarray(bo_re.reshape(8, 128).T.astype(np.float32))  # [128, 8]
    bo_im = np.ascontiguousarray(bo_im.reshape(8, 128).T.astype(np.float32))

    def mblk(M):  # [2C, C] -> [p, g, kt, 512] contiguous per partition
        Mb = M.astype(BF).reshape(KT, 128, 2, 512)
        return np.ascontiguousarray(Mb.transpose(1, 2, 0, 3))

    shared = dict(
        xb=xb, M_re=mblk(M_re), M_im=mblk(M_im), bo_re=bo_re, bo_im=bo_im
    )
    return shared, per_core


def _build_program():
    nc = bacc.Bacc("TRN2", target_bir_lowering=False, debug=False, num_devices=NCORES)

    # ---- DRAM I/O (host-blocked layouts: contiguous per partition) ----
    xb_d = nc.dram_tensor("xb", [128, B * NCH, KT, TF], BF16, kind="ExternalInput")
    wq_d = nc.dram_tensor("wq", [128, HPC, KT, 128], BF16, kind="ExternalInput")
    wk_d = nc.dram_tensor("wk", [128, HPC, KT, 128], BF16, kind="ExternalInput")
    wv_d = nc.dram_tensor("wv", [128, KT, 2 * HPC * D], BF16, kind="ExternalInput")
    bq_d = nc.dram_tensor("bq", [128, HPC], F32, kind="ExternalInput")
    Mre_d = nc.dram_tensor("M_re", [128, 2, KT, 512], BF16, kind="ExternalInput")
    Mim_d = nc.dram_tensor("M_im", [128, 2, KT, 512], BF16, kind="ExternalInput")
    bore_d = nc.dram_tensor("bo_re", [128, 8], F32, kind="ExternalInput")
    boim_d = nc.dram_tensor("bo_im", [128, 8], F32, kind="ExternalInput")
    # per-core output: rows = [re(1024); im(1024)], cols = [b0 slice | b1 slice]
    yout_d = nc.dram_tensor("yout", [2 * C, B * TSL], F32, kind="ExternalOutput")

    yout_t = yout_d.rearrange("(cb p) t -> p cb t", p=128)  # [128, 16, 512]

    with (
        tile.TileContext(nc) as tc,
        nc.allow_low_precision(
            reason="bf16 intermediates; rounding matches low-precision matmul noise"
        ),
    ):
        with tc.tile_pool(name="dram", bufs=1, space="DRAM") as dram:
            # A2A buffers: [dest rank, ch, TSL tok]; batch 0 whole, b1 per head
            outc_dr0 = dram.tile([NCORES, 256, TSL], BF16, name="outc0", tag="outc0")
            at_dr0 = dram.tile([NCORES, 256, TSL], BF16, name="at0d", tag="at0d")
            outc_dr1 = [
                dram.tile([NCORES, 128, TSL], BF16, name=f"outc1{h}", tag=f"outc1{h}")
                for h in range(HPC)
            ]
            at_dr1 = [
                dram.tile([NCORES, 128, TSL], BF16, name=f"at1{h}", tag=f"at1{h}")
                for h in range(HPC)
            ]

            with (
                tc.tile_pool(name="keep", bufs=1) as keep,
                # PSUM: pair(2 banks x2) + av(1 bank x2) + den(1 bank x2) = 8.
                tc.tile_pool(name="pair_ps", bufs=2, space="PSUM") as pair_ps,
                tc.tile_pool(name="av_ps", bufs=2, space="PSUM") as av_ps,
                tc.tile_pool(name="den_ps", bufs=2, space="PSUM") as den_ps,
                tc.tile_pool(name="qk_sb", bufs=1) as qk_sb,
                tc.tile_pool(name="v_sb", bufs=1) as v_sbp,
                tc.tile_pool(name="expp", bufs=4) as expp,
                tc.tile_pool(name="dtp", bufs=2) as dtp,
                tc.tile_pool(name="evp", bufs=3) as evp,
                tc.tile_pool(name="const", bufs=1) as const,
            ):
                ctx_x = ExitStack()
                xp = ctx_x.enter_context(tc.tile_pool(name="xp", bufs=2))

                # ---- startup: first x chunk split per-2kt across BOTH data
                # rings; weights per-kt on the gpsimd ring so the first chain
                # starts ~2us in ----
                xtiles = {}
                xt0 = xp.tile([128, KT, TF], BF16, name="xt", tag="xt")
                xtiles[0] = xt0
                for ks in range(0, KT, 2):
                    eng = nc.sync if (ks // 2) % 2 == 0 else nc.scalar
                    eng.dma_start(xt0[:, ks : ks + 2, :], xb_d[:, 0, ks : ks + 2, :])
                wq_sb = const.tile([128, HPC, KT, 128], BF16)
                wk_sb = const.tile([128, HPC, KT, 128], BF16)
                wv_sb = const.tile([128, KT, 2 * HPC * D], BF16)
                for kt in range(KT):
                    nc.scalar.dma_start(wq_sb[:, :, kt, :], wq_d[:, :, kt, :])
                nc.scalar.dma_start(wk_sb[:], wk_d[:])
                nc.gpsimd.dma_start(wv_sb[:], wv_d[:])
                bq_sb = keep.tile([128, HPC], F32)
                nc.gpsimd.dma_start(bq_sb[:], bq_d[:])
                if USE_FP8_AV:
                    # all-ones stationary for the DoubleRow denominator matmul
                    ones_t = keep.tile([128, 2, 128], FP8)
                else:
                    ones_t = keep.tile([128, 128], BF16)
                nc.vector.memset(ones_t[:], 1.0)
                # softmax shift (exact): score/8 tops out near +8.1, and
                # fp8e4m3 saturates at 448 — shift so max exp is ~e^4.7
                negone = keep.tile([128, 1], F32)
                nc.vector.memset(negone[:], -3.5)

                qc = {}
                kc = {}
                vt = {}
                for b in range(B):
                    for hh in range(HPC):
                        qc[b, hh] = qk_sb.tile(
                            [128, N], BF16, name=f"qc{b}{hh}", tag=f"qc{b}{hh}"
                        )
                        kc[b, hh] = qk_sb.tile(
                            [128, N], BF16, name=f"kc{b}{hh}", tag=f"kc{b}{hh}"
                        )
                        # fp8: [kp pair, 2, ch] layout feeds DoubleRow directly
                        vt[b, hh] = v_sbp.tile(
                            [128, N // 256, 2, 128] if USE_FP8_AV else [128, N // 128, 128],
                            FP8 if USE_FP8_AV else BF16,
                            name=f"vt{b}{hh}",
                            tag=f"vt{b}{hh}",
                        )

                def emit_x_dma(cg, split=False):
                    """Prefetch x chunk cg (global index). split: per-2kt
                    slices on the sync ring so early chains aren't starved
                    while the weight stream still owns the scalar ring."""
                    xt = xp.tile([128, KT, TF], BF16, name="xt", tag="xt")
                    xtiles[cg] = xt
                    if split:
                        for ks in range(0, KT, 2):
                            nc.sync.dma_start(
                                xt[:, ks : ks + 2, :], xb_d[:, cg, ks : ks + 2, :]
                            )
                    else:
                        eng = nc.scalar if cg % 2 == 0 else nc.sync
                        eng.dma_start(xt[:], xb_d[:, cg, :, :])

                def emit_proj_qk(b, ci):
                    """q and k chains for chunk ci of batch b."""
                    cg = b * NCH + ci
                    xt = xtiles[cg]
                    csl = slice(ci * TF, ci * TF + TF)
                    prs = {}
                    for hh in range(HPC):
                        prs[hh] = pair_ps.tile([128, 1024], F32, name="prps", tag="prps")
                        for kt in range(KT):
                            nc.tensor.matmul(
                                prs[hh][:, 0:512],
                                wq_sb[:, hh, kt, :],
                                xt[:, kt, :],
                                start=(kt == 0),
                                stop=(kt == KT - 1),
                            )
                        nc.scalar.activation(
                            qc[b, hh][:, csl],
                            prs[hh][:, 0:512],
                            mybir.ActivationFunctionType.Identity,
                            bias=bq_sb[:, hh : hh + 1],
                        )
                    for hh in range(HPC):
                        for kt in range(KT):
                            nc.tensor.matmul(
                                prs[hh][:, 512:1024],
                                wk_sb[:, hh, kt, :],
                                xt[:, kt, :],
                                start=(kt == 0),
                                stop=(kt == KT - 1),
                            )
                        nc.vector.tensor_copy(kc[b, hh][:, csl], prs[hh][:, 512:1024])

                def emit_proj_v(b, ci):
                    """v chains for chunk ci of batch b."""
                    cg = b * NCH + ci
                    xt = xtiles[cg]
                    vtiles = [
                        pair_ps.tile([128, 1024], F32, name="prps", tag="prps")
                        for _ in range(2)
                    ]
                    for m in range(TF // 128):
                        # alternate tiles and half-banks so the DVE read of
                        # chain m-1 never shares a bank with chain m's write
                        vp = vtiles[m % 2][:, (m // 2) * 512 : (m // 2) * 512 + 256]
                        for kt in range(KT):
                            nc.tensor.matmul(
                                vp,
                                xt[:, kt, m * 128 : (m + 1) * 128],
                                wv_sb[:, kt, :],
                                start=(kt == 0),
                                stop=(kt == KT - 1),
                            )
                        ktok = ci * (TF // 128) + m
                        for hh in range(HPC):
                            dst = (
                                vt[b, hh][:, ktok // 2, ktok % 2, :]
                                if USE_FP8_AV
                                else vt[b, hh][:, ktok, :]
                            )
                            nc.vector.tensor_copy(
                                dst, vp[:, hh * 128 : (hh + 1) * 128]
                            )

                deferred = [None]  # single-slot pipeline for softmax tails

                def flush_deferred():
                    if deferred[0] is not None:
                        deferred[0]()
                        deferred[0] = None

                def emit_attn_qt(b, hh, qt):
                    """Scores/exp/av for one 512-query tile; the softmax tail
                    (den matmuls, reciprocal, scale, store) is deferred into
                    the next tile so the PE never idles at tile boundaries."""
                    qsl = slice(qt * 512, (qt + 1) * 512)
                    av = av_ps.tile([128, 512], F32, name="avps", tag="avps")

                    def _av(kp, ex):
                        for j in range(2):
                            kt = 2 * kp + j
                            nc.tensor.matmul(
                                av[:],
                                vt[b, hh][:, kt, :],
                                ex[:, j, :],
                                start=(kt == 0),
                                stop=(kt == N // 128 - 1),
                            )

                    pend = []
                    exs = []
                    pairs = {}
                    quads = {}
                    for kp in range(NKP):
                        pr = pair_ps.tile([128, 1024], F32, name="prps", tag="prps")
                        for j in range(2):
                            kt = 2 * kp + j
                            nc.tensor.matmul(
                                pr[:, j * 512 : (j + 1) * 512],
                                kc[b, hh][:, kt * 128 : (kt + 1) * 128],
                                qc[b, hh][:, qsl],
                                start=True,
                                stop=True,
                            )
                        ex = expp.tile([128, 2, 512], BF16, name="ex", tag="ex")
                        nc.scalar.activation(
                            ex[:, :, :],
                            pr[:],
                            mybir.ActivationFunctionType.Exp,
                            scale=0.125,
                        )
                        exs.append(ex)
                        # denominator tree-adds on the idle DVE (bf16 2x mode)
                        if kp % 2 == 1:
                            p = kp // 2
                            pairs[p] = dtp.tile(
                                [128, 1024], BF16, name=f"dp{p % 2}", tag=f"dp{p % 2}"
                            )
                            nc.vector.tensor_tensor(
                                pairs[p][:], exs[kp - 1][:], ex[:], mybir.AluOpType.add
                            )
                        if kp % 4 == 3:
                            q4 = kp // 4
                            quads[q4] = dtp.tile(
                                [128, 1024], BF16, name=f"dq{q4}", tag=f"dq{q4}"
                            )
                            nc.vector.tensor_tensor(
                                quads[q4][:],
                                pairs[kp // 2 - 1][:],
                                pairs[kp // 2][:],
                                mybir.AluOpType.add,
                            )
                        pend.append((kp, ex))
                        if kp == 1:
                            # PE is 2 key-pairs into this tile: safe point to
                            # emit the previous tile's softmax tail
                            flush_deferred()
                        if len(pend) > 2:
                            _av(*pend.pop(0))
                    pend_tail = list(pend)

                    def tail():
                        # last two av accumulations land here so the PE never
                        # waits on the exp pipeline at the tile boundary
                        for item in pend_tail:
                            _av(*item)
                        den = den_ps.tile([128, 512], F32, name="denps", tag="denps")
                        for i4, q4t in enumerate((quads[0], quads[1])):
                            for j in range(2):
                                nc.tensor.matmul(
                                    den[:],
                                    ones_t[:],
                                    q4t[:, j * 512 : (j + 1) * 512],
                                    start=(i4 == 0 and j == 0),
                                    stop=(i4 == 1 and j == 1),
                                )
                        rb = evp.tile([128, 512], F32, name="rb", tag="rb")
                        nc.vector.reciprocal_approx_fast(rb[:], den[:])
                        outc = evp.tile([128, 512], BF16, name="outc", tag="outc")
                        nc.vector.tensor_tensor(
                            outc[:], av[:], rb[:], mybir.AluOpType.mult
                        )
                        for j in range(2):
                            if b == 0:
                                dst = outc_dr0[2 * qt + j, hh * 128 : (hh + 1) * 128, :]
                            else:
                                dst = outc_dr1[hh][2 * qt + j, :, :]
                            nc.sync.dma_start(dst, outc[:, j * TSL : (j + 1) * TSL])

                    deferred[0] = tail

                # ---- phase 1: projections for batch 0. Chunk prefetches are
                # emitted AFTER the consuming chains of the previous chunk so
                # they never jump the DMA line ahead of the weight stream ----
                for ci in range(NCH):
                    emit_proj_qk(0, ci)
                    if ci + 1 < NCH:
                        emit_x_dma(ci + 1, split=(ci == 0))
                    emit_proj_v(0, ci)
                emit_x_dma(NCH)  # first b1 chunk

                # ---- phase 2a: attention(b0) with HALF of proj(b1) woven in
                # (fills the exp-bound slack and keeps the PE at full clock);
                # the other half runs after the b0 A2A fires, hiding the
                # first collective's cross-core skew sync (~17us) ----
                for hh in range(HPC):
                    for qt in range(N // 512):
                        emit_attn_qt(0, hh, qt)
                        si = hh * 4 + qt
                        if si == 0:
                            emit_proj_qk(1, 0)
                        elif si == 1:
                            emit_x_dma(NCH + 1)
                            emit_proj_v(1, 0)
                        elif si == 2:
                            emit_proj_qk(1, 1)
                        elif si == 3:
                            emit_x_dma(NCH + 2)
                            emit_proj_v(1, 1)
                        elif si == 5:
                            emit_x_dma(NCH + 3)
                flush_deferred()
                nc.gpsimd.collective_compute(
                    "AllToAll",
                    mybir.AluOpType.bypass,
                    replica_groups=[list(range(NCORES))],
                    ins=[outc_dr0.opt()],
                    outs=[at_dr0.opt()],
                )

                # ---- phase 2b: rest of proj(b1) under the b0 exchange ----
                for ci in (2, 3):
                    emit_proj_qk(1, ci)
                    emit_proj_v(1, ci)

                # x pool no longer needed: free it for the o-phase M tiles
                ctx_x.close()
                opool = ctx_x.enter_context(tc.tile_pool(name="opool", bufs=1))
                oev = ctx_x.enter_context(tc.tile_pool(name="oev", bufs=2))
                m_tiles = []
                for g in range(2):
                    for part, M_d in ((0, Mre_d), (1, Mim_d)):
                        m_sb = opool.tile(
                            [128, KT, 512], BF16, name=f"m{g}{part}", tag=f"m{g}{part}"
                        )
                        eng = nc.scalar if part == 0 else nc.sync
                        eng.dma_start(m_sb[:], M_d[:, g, :, :])
                        m_tiles.append((g, part, m_sb))
                bo_sb = [None, None]
                bo_sb[0] = keep.tile([128, 8], F32, name="bo_re", tag="bo_re")
                bo_sb[1] = keep.tile([128, 8], F32, name="bo_im", tag="bo_im")
                nc.gpsimd.dma_start(bo_sb[0][:], bore_d[:])
                nc.gpsimd.dma_start(bo_sb[1][:], boim_d[:])
                at_sb = [None, None]
                at_sb[0] = opool.tile(
                    [128, HPC, NCORES, TSL], BF16, name="at0", tag="at0"
                )
                at_sb[1] = opool.tile(
                    [128, HPC, NCORES, TSL], BF16, name="at1", tag="at1"
                )
                # at0 load on the gpsimd queue right after the A2A it waits on
                at0_t = at_dr0.rearrange("r (hp p) t -> p hp r t", p=128)
                for h in range(HPC):  # DMA APs are limited to 3 dims
                    nc.gpsimd.dma_start(at_sb[0][:, h, :, :], at0_t[:, h, :, :])

                def emit_oproj_group(b, g, part, m_sb):
                    """One quarter of the o-projection for batch b: 4 chains
                    of 128 output channels x TSL tokens + bias + store."""
                    otiles = [
                        pair_ps.tile([128, 1024], F32, name="prps", tag="prps"),
                        pair_ps.tile([128, 1024], F32, name="prps", tag="prps"),
                    ]
                    y_sb = oev.tile([128, 4, TSL], F32, name="y_sb", tag="y_sb")
                    for i in range(4):
                        # alternate tiles/half-banks: ACT reads chain i-1's
                        # bank while the PE accumulates into another
                        ps = otiles[i % 2][:, (i // 2) * 512 : (i // 2) * 512 + TSL]
                        for kt in range(KT):
                            # contraction row kt = (src rank kt//2, head kt%2)
                            nc.tensor.matmul(
                                ps,
                                m_sb[:, kt, i * 128 : (i + 1) * 128],
                                at_sb[b][:, kt % 2, kt // 2, :],
                                start=(kt == 0),
                                stop=(kt == KT - 1),
                            )
                        nc.scalar.activation(
                            y_sb[:, i, :],
                            ps,
                            mybir.ActivationFunctionType.Identity,
                            bias=bo_sb[part][:, g * 4 + i : g * 4 + i + 1],
                        )
                    cb0 = part * 8 + g * 4
                    nc.sync.dma_start(
                        yout_t[:, cb0 : cb0 + 4, b * TSL : (b + 1) * TSL], y_sb[:]
                    )

                # ---- phase 3: attention(b1) interleaved with o-proj(b0).
                # Groups go after mid-head qts only: a group after the last
                # qt of a head would delay that head's outc stores and A2A ----
                og = []
                for hh in range(HPC):
                    for qt in range(N // 512):
                        emit_attn_qt(1, hh, qt)
                        if (hh, qt) in og:
                            gi = og.index((hh, qt))
                            emit_oproj_group(0, *m_tiles[gi][:2], m_tiles[gi][2])
                    flush_deferred()
                    # head hh's rows ship while the next head computes
                    nc.gpsimd.collective_compute(
                        "AllToAll",
                        mybir.AluOpType.bypass,
                        replica_groups=[list(range(NCORES))],
                        ins=[outc_dr1[hh].opt()],
                        outs=[at_dr1[hh].opt()],
                    )

                # ---- phase 4: o-proj(b0) under the last A2A ----
                for gi in (0, 1, 2, 3):
                    emit_oproj_group(0, *m_tiles[gi][:2], m_tiles[gi][2])
                for h in range(HPC):
                    at1h_t = at_dr1[h].rearrange("r p t -> p r t")
                    nc.gpsimd.dma_start(at_sb[1][:, h, :, :], at1h_t[:, :, :])
                for gi in range(4):
                    emit_oproj_group(1, *m_tiles[gi][:2], m_tiles[gi][2])
                ctx_x.close()  # opool/oev close before the outer pools (LIFO)
    nc.compile()
    return nc


_NC_CACHE = None


def _get_program():
    global _NC_CACHE
    if _NC_CACHE is None:
        _NC_CACHE = _build_program()
    return _NC_CACHE


def _run(inputs, trace=False, trace_kwargs=None):
    shared, per_core = _host_prep(inputs)
    nc = _get_program()
    in_maps = []
    for c in range(NCORES):
        d = per_core[c]
        in_maps.append(
            {
                "xb": shared["xb"],
                "wq": d["wq"],
                "wk": d["wk"],
                "wv": d["wv"],
                "bq": d["bq"],
                "M_re": shared["M_re"],
                "M_im": shared["M_im"],
                "bo_re": shared["bo_re"],
                "bo_im": shared["bo_im"],
            }
        )
    res = run_bass_kernel_spmd(
        nc, in_maps, list(range(NCORES)), trace=trace, **(trace_kwargs or {})
    )
    youts = [res.results[c]["yout"] for c in range(NCORES)]
    # youts[c]: [2C, B*TSL]; rows [re(1024); im(1024)], cols [b0 256 | b1 256]
    re = np.zeros((B, N, C), dtype=np.float32)
    im = np.zeros((B, N, C), dtype=np.float32)
    for c in range(NCORES):
        for b in range(B):
            tsl = slice(c * TSL, (c + 1) * TSL)
            re[b, tsl] = youts[c][:C, b * TSL : (b + 1) * TSL].T
            im[b, tsl] = youts[c][C:, b * TSL : (b + 1) * TSL].T
    return np.stack([re, im]).astype(np.float32), res


def kernel(**inputs) -> np.ndarray:
    out, _ = _run(inputs, trace=False)
    return out


# revision 35
# speedup vs baseline: 1.0812x; 1.0097x over previous
"""Complex self-attention on 8 Trainium2 NeuronCores (Bass/Tile).

Model (reference): complex linear q/k/v projections of (x_re, x_im), attention
scores = (Re(q)·Re(k) + Im(q)·Im(k))/sqrt(D), softmax, attn applied to Re(v)
and Im(v), complex output projection. B=2, N=2048, C=1024, H=16, D=64.

Sharding: heads tensor-parallel across 8 cores (2 heads/core, both batches).
  - Projections: stacked-contraction trick ([x_re; x_im], 2C rows) against
    host-combined weights — each complex part is ONE bf16 matmul chain.
  - Software-pipelined phase schedule keeps the PE continuously busy (TRN2
    drops to half clock for 3us after ANY idle gap):
      phase 1: proj(b0)
      phase 2: attention(b0) interleaved with proj(b1)  [exp-bound slots
               filled with projection chains]
      phase 3: attention(b1) interleaved with half of o-proj(b0)
      phase 4: rest of o-proj(b0) under the last A2A, then o-proj(b1)
    The softmax tail of each query tile (den matmuls, reciprocal, scale,
    store) is deferred into the NEXT tile's emission so the PE never waits
    for the exp/DVE pipeline at tile boundaries.
  - Softmax denominator: exp tiles are tree-summed on the idle DVE in bf16,
    then 4 ones-matmuls per query tile (instead of 16) do the partition sum.
  - 1/den via reciprocal_approx_fast (~5x faster, 18-bit accurate).
  - k bias dropped (exact by softmax shift invariance); v bias folded into
    the o-projection bias on the host (softmax rows sum to 1).
  - A2A: batch 0 in one exchange (hidden under phase 3), batch 1 per head so
    only the last 0.5 MB exchange is exposed.
"""

import sys

if "/opt/trn_rl_repo" not in sys.path:
    sys.path.insert(0, "/opt/trn_rl_repo")

from contextlib import ExitStack

import ml_dtypes
import numpy as np

import concourse.mybir as mybir
import concourse.tile as tile
from concourse import bacc
from concourse.bass_utils import run_bass_kernel_spmd

B, N, C = 2, 2048, 1024
H, D = 16, 64
T = B * N  # 4096 tokens total
NCORES = 8
HPC = H // NCORES  # 2 heads per core
TSL = N // NCORES  # 256-token output slice per core PER BATCH
TF = 512  # projection token-chunk (free dim)
KT = 2 * C // 128  # 16 contraction tiles of 128 over [x_re; x_im]
NCH = N // TF  # token chunks per batch (4)
NKP = N // 256  # key-tile PAIRS per query tile (8): each pair = 2 x 128 keys
F32 = mybir.dt.float32
BF16 = mybir.dt.bfloat16
FP8 = mybir.dt.float8e4
BF = ml_dtypes.bfloat16

# fp8e4m3 exp/v with DoubleRow av+den measured FASTER on paper but fails the
# correctness gate (2.8e-2 rel err: weighted-sum quantization noise does not
# average down) and LDWEIGHTS-bound in practice. Keep the bf16 path.
USE_FP8_AV = False


def _host_prep(inp):
    """Build the host-side sharded/combined arrays (all matmul inputs bf16,
    blocked so every DMA is contiguous per partition)."""
    x_re = np.asarray(inp["x_re"], dtype=np.float32).reshape(T, C)
    x_im = np.asarray(inp["x_im"], dtype=np.float32).reshape(T, C)
    xT2 = np.concatenate([x_re.T, x_im.T], axis=0).astype(BF)  # [2C, T]
    # blocked: [p, chunk, kt, t] so a chunk load is 16KB contiguous/partition
    xb = np.ascontiguousarray(
        xT2.reshape(KT, 128, B * NCH, TF).transpose(1, 2, 0, 3)
    )

    per_core = []
    for c in range(NCORES):
        d = {}
        h0 = c * HPC
        ch = slice(h0 * D, (h0 + HPC) * D)
        for nm in ("q", "k", "v"):
            Wre = np.asarray(inp[f"{nm}_Wre"], dtype=np.float32)[ch]  # [128, C]
            Wim = np.asarray(inp[f"{nm}_Wim"], dtype=np.float32)[ch]
            bre = np.asarray(inp[f"{nm}_bre"], dtype=np.float32)[ch]
            bim = np.asarray(inp[f"{nm}_bim"], dtype=np.float32)[ch]
            Ws, bs = [], []
            for hh in range(HPC):
                hs = slice(hh * D, (hh + 1) * D)
                wr = np.concatenate([Wre[hs].T, -Wim[hs].T], axis=0)  # [2C, 64]
                wi = np.concatenate([Wim[hs].T, Wre[hs].T], axis=0)
                Ws.append(np.concatenate([wr, wi], axis=1))  # [2C, 128]
                bs.append(np.concatenate([bre[hs] - bim[hs], bre[hs] + bim[hs]]))
            if nm == "v":
                wvb = np.concatenate(Ws, axis=1).astype(BF)  # [2C, 256]
                d["wv"] = np.ascontiguousarray(
                    wvb.reshape(KT, 128, 2 * HPC * D).transpose(1, 0, 2)
                )  # [p, kt, 256]
            else:
                wqk = np.stack(Ws).astype(BF)  # [HPC, 2C, 128]
                d[f"w{nm}"] = np.ascontiguousarray(
                    wqk.reshape(HPC, KT, 128, 128).transpose(2, 0, 1, 3)
                )  # [p, hh, kt, m]
                if nm == "q":
                    d["bq"] = np.ascontiguousarray(np.stack(bs, axis=1))  # [128, HPC]
        per_core.append(d)

    # o-projection combined matrices, rows ordered to match the A2A result:
    # src rank r, then per rank [h0:out_r(64), h0:out_i(64), h1:..., h1:...]
    oWre = np.asarray(inp["o_Wre"], dtype=np.float32)
    oWim = np.asarray(inp["o_Wim"], dtype=np.float32)
    Mre_rows, Mim_rows = [], []
    bv_rows = []
    vbre = np.asarray(inp["v_bre"], dtype=np.float32)
    vbim = np.asarray(inp["v_bim"], dtype=np.float32)
    for r in range(NCORES):
        for hh in range(HPC):
            h = r * HPC + hh
            hs = slice(h * D, (h + 1) * D)
            Mre_rows += [oWre[:, hs].T, -oWim[:, hs].T]
            Mim_rows += [oWim[:, hs].T, oWre[:, hs].T]
            bv_rows += [vbre[hs] - vbim[hs], vbre[hs] + vbim[hs]]
    M_re = np.concatenate(Mre_rows, axis=0)  # [2C, C]
    M_im = np.concatenate(Mim_rows, axis=0)
    bv_full = np.concatenate(bv_rows)  # [2C] — v bias in A2A row order
    o_bre = np.asarray(inp["o_bre"], dtype=np.float32)
    o_bim = np.asarray(inp["o_bim"], dtype=np.float32)
    # fold the v bias through the o-projection (softmax rows sum to 1)
    bo_re = (o_bre - o_bim) + M_re.T @ bv_full  # [C]
    bo_im = (o_bre + o_bim) + M_im.T @ bv_full
    bo_re = np.ascontiguousarray(bo_re.reshape(8, 128).T.astype(np.float32))  # [128, 8]
    bo_im = np.ascontiguousarray(bo_im.reshape(8, 128).T.astype(np.float32))

    def mblk(M):  # [2C, C] -> [p, g, kt, 512] contiguous per partition
        Mb = M.astype(BF).reshape(KT, 128, 2, 512)
        return np.ascontiguousarray(Mb.transpose(1, 2, 0, 3))

    shared = dict(
        xb=xb, M_re=mblk(M_re), M_im=mblk(M_im), bo_re=bo_re, bo_im=bo_im
    )
    return shared, per_core


def _build_program():
    nc = bacc.Bacc("TRN2", target_bir_lowering=False, debug=False, num_devices=NCORES)

    # ---- DRAM I/O (host-blocked layouts: contiguous per partition) ----
    xb_d = nc.dram_tensor("xb", [128, B * NCH, KT, TF], BF16, kind="ExternalInput")
    wq_d = nc.dram_tensor("wq", [128, HPC, KT, 128], BF16, kind="ExternalInput")
    wk_d = nc.dram_tensor("wk", [128, HPC, KT, 128], BF16, kind="ExternalInput")
    wv_d = nc.dram_tensor("wv", [128, KT, 2 * HPC * D], BF16, kind="ExternalInput")
    bq_d = nc.dram_tensor("bq", [128, HPC], F32, kind="ExternalInput")
    Mre_d = nc.dram_tensor("M_re", [128, 2, KT, 512], BF16, kind="ExternalInput")
    Mim_d = nc.dram_tensor("M_im", [128, 2, KT, 512], BF16, kind="ExternalInput")
    bore_d = nc.dram_tensor("bo_re", [128, 8], F32, kind="ExternalInput")
    boim_d = nc.dram_tensor("bo_im", [128, 8], F32, kind="ExternalInput")
    # per-core output: rows = [re(1024); im(1024)], cols = [b0 slice | b1 slice]
    yout_d = nc.dram_tensor("yout", [2 * C, B * TSL], F32, kind="ExternalOutput")

    yout_t = yout_d.rearrange("(cb p) t -> p cb t", p=128)  # [128, 16, 512]

    with (
        tile.TileContext(nc) as tc,
        nc.allow_low_precision(
            reason="bf16 intermediates; rounding matches low-precision matmul noise"
        ),
    ):
        with tc.tile_pool(name="dram", bufs=1, space="DRAM") as dram:
            # A2A buffers: [dest rank, ch, TSL tok]; batch 0 whole, b1 per head
            outc_dr0 = dram.tile([NCORES, 256, TSL], BF16, name="outc0", tag="outc0")
            at_dr0 = dram.tile([NCORES, 256, TSL], BF16, name="at0d", tag="at0d")
            outc_dr1 = [
                dram.tile([NCORES, 128, TSL], BF16, name=f"outc1{h}", tag=f"outc1{h}")
                for h in range(HPC)
            ]
            at_dr1 = [
                dram.tile([NCORES, 128, TSL], BF16, name=f"at1{h}", tag=f"at1{h}")
                for h in range(HPC)
            ]

            with (
                tc.tile_pool(name="keep", bufs=1) as keep,
                # PSUM: pair(2 banks x2) + av(1 bank x2) + den(1 bank x2) = 8.
                tc.tile_pool(name="pair_ps", bufs=2, space="PSUM") as pair_ps,
                tc.tile_pool(name="av_ps", bufs=2, space="PSUM") as av_ps,
                tc.tile_pool(name="den_ps", bufs=2, space="PSUM") as den_ps,
                tc.tile_pool(name="qk_sb", bufs=1) as qk_sb,
                tc.tile_pool(name="v_sb", bufs=1) as v_sbp,
                tc.tile_pool(name="expp", bufs=4) as expp,
                tc.tile_pool(name="dtp", bufs=2) as dtp,
                tc.tile_pool(name="evp", bufs=3) as evp,
                tc.tile_pool(name="const", bufs=1) as const,
            ):
                ctx_x = ExitStack()
                xp = ctx_x.enter_context(tc.tile_pool(name="xp", bufs=2))

                # ---- startup: first x chunk split per-2kt across BOTH data
                # rings; weights per-kt on the gpsimd ring so the first chain
                # starts ~2us in ----
                xtiles = {}
                xt0 = xp.tile([128, KT, TF], BF16, name="xt", tag="xt")
                xtiles[0] = xt0
                for ks in range(0, KT, 2):
                    eng = nc.sync if (ks // 2) % 2 == 0 else nc.scalar
                    eng.dma_start(xt0[:, ks : ks + 2, :], xb_d[:, 0, ks : ks + 2, :])
                wq_sb = const.tile([128, HPC, KT, 128], BF16)
                wk_sb = const.tile([128, HPC, KT, 128], BF16)
                wv_sb = const.tile([128, KT, 2 * HPC * D], BF16)
                for kt in range(KT):
                    nc.scalar.dma_start(wq_sb[:, :, kt, :], wq_d[:, :, kt, :])
                nc.scalar.dma_start(wk_sb[:], wk_d[:])
                nc.gpsimd.dma_start(wv_sb[:], wv_d[:])
                bq_sb = keep.tile([128, HPC], F32)
                nc.gpsimd.dma_start(bq_sb[:], bq_d[:])
                if USE_FP8_AV:
                    # all-ones stationary for the DoubleRow denominator matmul
                    ones_t = keep.tile([128, 2, 128], FP8)
                else:
                    ones_t = keep.tile([128, 128], BF16)
                nc.vector.memset(ones_t[:], 1.0)
                # softmax shift (exact): score/8 tops out near +8.1, and
                # fp8e4m3 saturates at 448 — shift so max exp is ~e^4.7
                negone = keep.tile([128, 1], F32)
                nc.vector.memset(negone[:], -3.5)

                qc = {}
                kc = {}
                vt = {}
                for b in range(B):
                    for hh in range(HPC):
                        qc[b, hh] = qk_sb.tile(
                            [128, N], BF16, name=f"qc{b}{hh}", tag=f"qc{b}{hh}"
                        )
                        kc[b, hh] = qk_sb.tile(
                            [128, N], BF16, name=f"kc{b}{hh}", tag=f"kc{b}{hh}"
                        )
                        # fp8: [kp pair, 2, ch] layout feeds DoubleRow directly
                        vt[b, hh] = v_sbp.tile(
                            [128, N // 256, 2, 128] if USE_FP8_AV else [128, N // 128, 128],
                            FP8 if USE_FP8_AV else BF16,
                            name=f"vt{b}{hh}",
                            tag=f"vt{b}{hh}",
                        )

                def emit_x_dma(cg, split=False):
                    """Prefetch x chunk cg (global index). split: per-2kt
                    slices on the sync ring so early chains aren't starved
                    while the weight stream still owns the scalar ring."""
                    xt = xp.tile([128, KT, TF], BF16, name="xt", tag="xt")
                    xtiles[cg] = xt
                    if split:
                        for ks in range(0, KT, 2):
                            nc.sync.dma_start(
                                xt[:, ks : ks + 2, :], xb_d[:, cg, ks : ks + 2, :]
                            )
                    else:
                        eng = nc.scalar if cg % 2 == 0 else nc.sync
                        eng.dma_start(xt[:], xb_d[:, cg, :, :])

                def emit_proj_qk(b, ci):
                    """q and k chains for chunk ci of batch b."""
                    cg = b * NCH + ci
                    xt = xtiles[cg]
                    csl = slice(ci * TF, ci * TF + TF)
                    prs = {}
                    for hh in range(HPC):
                        prs[hh] = pair_ps.tile([128, 1024], F32, name="prps", tag="prps")
                        for kt in range(KT):
                            nc.tensor.matmul(
                                prs[hh][:, 0:512],
                                wq_sb[:, hh, kt, :],
                                xt[:, kt, :],
                                start=(kt == 0),
                                stop=(kt == KT - 1),
                            )
                        nc.scalar.activation(
                            qc[b, hh][:, csl],
                            prs[hh][:, 0:512],
                            mybir.ActivationFunctionType.Identity,
                            bias=bq_sb[:, hh : hh + 1],
                        )
                    for hh in range(HPC):
                        for kt in range(KT):
                            nc.tensor.matmul(
                                prs[hh][:, 512:1024],
                                wk_sb[:, hh, kt, :],
                                xt[:, kt, :],
                                start=(kt == 0),
                                stop=(kt == KT - 1),
                            )
                        # split the drains across DVE and the (idle) scalar
                        # engine so the PSUM frees sooner
                        if hh == 0:
                            nc.vector.tensor_copy(kc[b, hh][:, csl], prs[hh][:, 512:1024])
                        else:
                            nc.scalar.copy(kc[b, hh][:, csl], prs[hh][:, 512:1024])

                def emit_proj_v(b, ci, borrow=False):
                    """v chains for chunk ci of batch b. borrow: attention is
                    not running, so rotate through the av/den PSUM banks too
                    (doubles the drain lead time ahead of each chain)."""
                    cg = b * NCH + ci
                    xt = xtiles[cg]
                    if borrow:
                        vps = [
                            av_ps.tile([128, 512], F32, name="avps", tag="avps")[:, 0:256],
                            den_ps.tile([128, 512], F32, name="denps", tag="denps")[:, 0:256],
                            av_ps.tile([128, 512], F32, name="avps", tag="avps")[:, 0:256],
                            den_ps.tile([128, 512], F32, name="denps", tag="denps")[:, 0:256],
                        ]
                    else:
                        vtiles = [
                            pair_ps.tile([128, 1024], F32, name="prps", tag="prps")
                            for _ in range(2)
                        ]
                        vps = [
                            vtiles[m % 2][:, (m // 2) * 512 : (m // 2) * 512 + 256]
                            for m in range(4)
                        ]
                    for m in range(TF // 128):
                        vp = vps[m]
                        for kt in range(KT):
                            nc.tensor.matmul(
                                vp,
                                xt[:, kt, m * 128 : (m + 1) * 128],
                                wv_sb[:, kt, :],
                                start=(kt == 0),
                                stop=(kt == KT - 1),
                            )
                        ktok = ci * (TF // 128) + m
                        for hh in range(HPC):
                            dst = (
                                vt[b, hh][:, ktok // 2, ktok % 2, :]
                                if USE_FP8_AV
                                else vt[b, hh][:, ktok, :]
                            )
                            if hh == 0:
                                nc.vector.tensor_copy(
                                    dst, vp[:, hh * 128 : (hh + 1) * 128]
                                )
                            else:
                                nc.scalar.copy(dst, vp[:, hh * 128 : (hh + 1) * 128])

                deferred = [None]  # single-slot pipeline for softmax tails

                def flush_deferred():
                    if deferred[0] is not None:
                        deferred[0]()
                        deferred[0] = None

                def emit_attn_qt(b, hh, qt, filler=None):
                    """Scores/exp/av for one 512-query tile; the softmax tail
                    (den matmuls, reciprocal, scale, store) is deferred into
                    the next tile so the PE never idles at tile boundaries.
                    filler: independent PE work emitted mid-tile (kp==3),
                    where the PSUM rotation has maximum slack."""
                    qsl = slice(qt * 512, (qt + 1) * 512)
                    av = av_ps.tile([128, 512], F32, name="avps", tag="avps")

                    def _av(kp, ex):
                        for j in range(2):
                            kt = 2 * kp + j
                            nc.tensor.matmul(
                                av[:],
                                vt[b, hh][:, kt, :],
                                ex[:, j, :],
                                start=(kt == 0),
                                stop=(kt == N // 128 - 1),
                            )

                    pend = []
                    exs = []
                    pairs = {}
                    quads = {}
                    for kp in range(NKP):
                        pr = pair_ps.tile([128, 1024], F32, name="prps", tag="prps")
                        for j in range(2):
                            kt = 2 * kp + j
                            nc.tensor.matmul(
                                pr[:, j * 512 : (j + 1) * 512],
                                kc[b, hh][:, kt * 128 : (kt + 1) * 128],
                                qc[b, hh][:, qsl],
                                start=True,
                                stop=True,
                            )
                        ex = expp.tile([128, 2, 512], BF16, name="ex", tag="ex")
                        nc.scalar.activation(
                            ex[:, :, :],
                            pr[:],
                            mybir.ActivationFunctionType.Exp,
                            scale=0.125,
                        )
                        exs.append(ex)
                        # denominator tree-adds on the idle DVE (bf16 2x mode)
                        if kp % 2 == 1:
                            p = kp // 2
                            pairs[p] = dtp.tile(
                                [128, 1024], BF16, name=f"dp{p % 2}", tag=f"dp{p % 2}"
                            )
                            nc.vector.tensor_tensor(
                                pairs[p][:], exs[kp - 1][:], ex[:], mybir.AluOpType.add
                            )
                        if kp % 4 == 3:
                            q4 = kp // 4
                            quads[q4] = dtp.tile(
                                [128, 1024], BF16, name=f"dq{q4}", tag=f"dq{q4}"
                            )
                            nc.vector.tensor_tensor(
                                quads[q4][:],
                                pairs[kp // 2 - 1][:],
                                pairs[kp // 2][:],
                                mybir.AluOpType.add,
                            )
                        pend.append((kp, ex))
                        if kp == 1:
                            # PE is 2 key-pairs into this tile: safe point to
                            # emit the previous tile's softmax tail
                            flush_deferred()
                        if kp == 3 and filler is not None:
                            filler()
                        if len(pend) > 2:
                            _av(*pend.pop(0))
                    pend_tail = list(pend)

                    def tail():
                        # last two av accumulations land here so the PE never
                        # waits on the exp pipeline at the tile boundary
                        for item in pend_tail:
                            _av(*item)
                        den = den_ps.tile([128, 512], F32, name="denps", tag="denps")
                        for i4, q4t in enumerate((quads[0], quads[1])):
                            for j in range(2):
                                nc.tensor.matmul(
                                    den[:],
                                    ones_t[:],
                                    q4t[:, j * 512 : (j + 1) * 512],
                                    start=(i4 == 0 and j == 0),
                                    stop=(i4 == 1 and j == 1),
                                )
                        rb = evp.tile([128, 512], F32, name="rb", tag="rb")
                        nc.vector.reciprocal_approx_fast(rb[:], den[:])
                        outc = evp.tile([128, 512], BF16, name="outc", tag="outc")
                        nc.vector.tensor_tensor(
                            outc[:], av[:], rb[:], mybir.AluOpType.mult
                        )
                        for j in range(2):
                            if b == 0:
                                dst = outc_dr0[2 * qt + j, hh * 128 : (hh + 1) * 128, :]
                            else:
                                dst = outc_dr1[hh][2 * qt + j, :, :]
                            nc.sync.dma_start(dst, outc[:, j * TSL : (j + 1) * TSL])

                    deferred[0] = tail

                # ---- phase 1: projections for batch 0. Chunk prefetches are
                # emitted AFTER the consuming chains of the previous chunk so
                # they never jump the DMA line ahead of the weight stream ----
                for ci in range(NCH):
                    emit_proj_qk(0, ci)
                    if ci + 1 < NCH:
                        emit_x_dma(ci + 1, split=(ci == 0))
                    emit_proj_v(0, ci, borrow=True)
                emit_x_dma(NCH)  # first b1 chunk

                # ---- phase 2a: attention(b0) with HALF of proj(b1) woven in
                # (fills the exp-bound slack and keeps the PE at full clock);
                # the other half runs after the b0 A2A fires, hiding the
                # first collective's cross-core skew sync (~17us) ----
                def _f(si):
                    if si == 0:
                        return lambda: emit_proj_qk(1, 0)
                    if si == 1:
                        return lambda: (emit_x_dma(NCH + 1), emit_proj_v(1, 0))
                    if si == 2:
                        return lambda: emit_proj_qk(1, 1)
                    if si == 3:
                        return lambda: (emit_x_dma(NCH + 2), emit_proj_v(1, 1))
                    if si == 5:
                        return lambda: emit_x_dma(NCH + 3)
                    return None

                for hh in range(HPC):
                    for qt in range(N // 512):
                        emit_attn_qt(0, hh, qt, filler=_f(hh * 4 + qt))
                flush_deferred()
                nc.gpsimd.collective_compute(
                    "AllToAll",
                    mybir.AluOpType.bypass,
                    replica_groups=[list(range(NCORES))],
                    ins=[outc_dr0.opt()],
                    outs=[at_dr0.opt()],
                )

                # ---- phase 2b: rest of proj(b1) under the b0 exchange ----
                for ci in (2, 3):
                    emit_proj_qk(1, ci)
                    emit_proj_v(1, ci, borrow=True)

                # x pool no longer needed: free it for the o-phase M tiles
                ctx_x.close()
                opool = ctx_x.enter_context(tc.tile_pool(name="opool", bufs=1))
                oev = ctx_x.enter_context(tc.tile_pool(name="oev", bufs=2))
                m_tiles = []
                for g in range(2):
                    for part, M_d in ((0, Mre_d), (1, Mim_d)):
                        m_sb = opool.tile(
                            [128, KT, 512], BF16, name=f"m{g}{part}", tag=f"m{g}{part}"
                        )
                        eng = nc.scalar if part == 0 else nc.sync
                        eng.dma_start(m_sb[:], M_d[:, g, :, :])
                        m_tiles.append((g, part, m_sb))
                bo_sb = [None, None]
                bo_sb[0] = keep.tile([128, 8], F32, name="bo_re", tag="bo_re")
                bo_sb[1] = keep.tile([128, 8], F32, name="bo_im", tag="bo_im")
                nc.gpsimd.dma_start(bo_sb[0][:], bore_d[:])
                nc.gpsimd.dma_start(bo_sb[1][:], boim_d[:])
                at_sb = [None, None]
                at_sb[0] = opool.tile(
                    [128, HPC, NCORES, TSL], BF16, name="at0", tag="at0"
                )
                at_sb[1] = opool.tile(
                    [128, HPC, NCORES, TSL], BF16, name="at1", tag="at1"
                )
                # at0 load on the gpsimd queue right after the A2A it waits on
                at0_t = at_dr0.rearrange("r (hp p) t -> p hp r t", p=128)
                for h in range(HPC):  # DMA APs are limited to 3 dims
                    nc.gpsimd.dma_start(at_sb[0][:, h, :, :], at0_t[:, h, :, :])

                def emit_oproj_group(b, g, part, m_sb):
                    """One quarter of the o-projection for batch b: 4 chains
                    of 128 output channels x TSL tokens + bias + store."""
                    otiles = [
                        pair_ps.tile([128, 1024], F32, name="prps", tag="prps"),
                        pair_ps.tile([128, 1024], F32, name="prps", tag="prps"),
                    ]
                    y_sb = oev.tile([128, 4, TSL], F32, name="y_sb", tag="y_sb")
                    for i in range(4):
                        # alternate tiles/half-banks: ACT reads chain i-1's
                        # bank while the PE accumulates into another
                        ps = otiles[i % 2][:, (i // 2) * 512 : (i // 2) * 512 + TSL]
                        for kt in range(KT):
                            # contraction row kt = (src rank kt//2, head kt%2)
                            nc.tensor.matmul(
                                ps,
                                m_sb[:, kt, i * 128 : (i + 1) * 128],
                                at_sb[b][:, kt % 2, kt // 2, :],
                                start=(kt == 0),
                                stop=(kt == KT - 1),
                            )
                        nc.scalar.activation(
                            y_sb[:, i, :],
                            ps,
                            mybir.ActivationFunctionType.Identity,
                            bias=bo_sb[part][:, g * 4 + i : g * 4 + i + 1],
                        )
                    cb0 = part * 8 + g * 4
                    nc.sync.dma_start(
                        yout_t[:, cb0 : cb0 + 4, b * TSL : (b + 1) * TSL], y_sb[:]
                    )

                # ---- phase 3: attention(b1) interleaved with o-proj(b0).
                # Groups go after mid-head qts only: a group after the last
                # qt of a head would delay that head's outc stores and A2A ----
                og = []
                for hh in range(HPC):
                    for qt in range(N // 512):
                        emit_attn_qt(1, hh, qt)
                        if (hh, qt) in og:
                            gi = og.index((hh, qt))
                            emit_oproj_group(0, *m_tiles[gi][:2], m_tiles[gi][2])
                    flush_deferred()
                    # head hh's rows ship while the next head computes; its
                    # at-load queues right behind the exchange on gpsimd
                    nc.gpsimd.collective_compute(
                        "AllToAll",
                        mybir.AluOpType.bypass,
                        replica_groups=[list(range(NCORES))],
                        ins=[outc_dr1[hh].opt()],
                        outs=[at_dr1[hh].opt()],
                    )
                    at1h_t = at_dr1[hh].rearrange("r p t -> p r t")
                    nc.gpsimd.dma_start(at_sb[1][:, hh, :, :], at1h_t[:, :, :])

                # ---- phase 4: o-proj(b0) under the last A2A ----
                for gi in (0, 1, 2, 3):
                    emit_oproj_group(0, *m_tiles[gi][:2], m_tiles[gi][2])
                for gi in range(4):
                    emit_oproj_group(1, *m_tiles[gi][:2], m_tiles[gi][2])
                ctx_x.close()  # opool/oev close before the outer pools (LIFO)
    nc.compile()
    return nc


_NC_CACHE = None


def _get_program():
    global _NC_CACHE
    if _NC_CACHE is None:
        _NC_CACHE = _build_program()
    return _NC_CACHE


def _run(inputs, trace=False, trace_kwargs=None):
    shared, per_core = _host_prep(inputs)
    nc = _get_program()
    in_maps = []
    for c in range(NCORES):
        d = per_core[c]
        in_maps.append(
            {
                "xb": shared["xb"],
                "wq": d["wq"],
                "wk": d["wk"],
                "wv": d["wv"],
                "bq": d["bq"],
                "M_re": shared["M_re"],
                "M_im": shared["M_im"],
                "bo_re": shared["bo_re"],
                "bo_im": shared["bo_im"],
            }
        )
    res = run_bass_kernel_spmd(
        nc, in_maps, list(range(NCORES)), trace=trace, **(trace_kwargs or {})
    )
    youts = [res.results[c]["yout"] for c in range(NCORES)]
    # youts[c]: [2C, B*TSL]; rows [re(1024); im(1024)], cols [b0 256 | b1 256]
    re = np.zeros((B, N, C), dtype=np.float32)
    im = np.zeros((B, N, C), dtype=np.float32)
    for c in range(NCORES):
        for b in range(B):
            tsl = slice(c * TSL, (c + 1) * TSL)
            re[b, tsl] = youts[c][:C, b * TSL : (b + 1) * TSL].T
            im[b, tsl] = youts[c][C:, b * TSL : (b + 1) * TSL].T
    return np.stack([re, im]).astype(np.float32), res


def kernel(**inputs) -> np.ndarray:
    out, _ = _run(inputs, trace=False)
    return out


# revision 36
# speedup vs baseline: 1.0949x; 1.0126x over previous
"""Complex self-attention on 8 Trainium2 NeuronCores (Bass/Tile).

Model (reference): complex linear q/k/v projections of (x_re, x_im), attention
scores = (Re(q)·Re(k) + Im(q)·Im(k))/sqrt(D), softmax, attn applied to Re(v)
and Im(v), complex output projection. B=2, N=2048, C=1024, H=16, D=64.

Sharding: heads tensor-parallel across 8 cores (2 heads/core, both batches).
  - Projections: stacked-contraction trick ([x_re; x_im], 2C rows) against
    host-combined weights — each complex part is ONE bf16 matmul chain.
  - Software-pipelined phase schedule keeps the PE continuously busy (TRN2
    drops to half clock for 3us after ANY idle gap):
      phase 1: proj(b0)
      phase 2: attention(b0) interleaved with proj(b1)  [exp-bound slots
               filled with projection chains]
      phase 3: attention(b1) interleaved with half of o-proj(b0)
      phase 4: rest of o-proj(b0) under the last A2A, then o-proj(b1)
    The softmax tail of each query tile (den matmuls, reciprocal, scale,
    store) is deferred into the NEXT tile's emission so the PE never waits
    for the exp/DVE pipeline at tile boundaries.
  - Softmax denominator: exp tiles are tree-summed on the idle DVE in bf16,
    then 4 ones-matmuls per query tile (instead of 16) do the partition sum.
  - 1/den via reciprocal_approx_fast (~5x faster, 18-bit accurate).
  - k bias dropped (exact by softmax shift invariance); v bias folded into
    the o-projection bias on the host (softmax rows sum to 1).
  - A2A: batch 0 in one exchange (hidden under phase 3), batch 1 per head so
    only the last 0.5 MB exchange is exposed.
"""

import sys

if "/opt/trn_rl_repo" not in sys.path:
    sys.path.insert(0, "/opt/trn_rl_repo")

from contextlib import ExitStack

import ml_dtypes
import numpy as np

import concourse.mybir as mybir
import concourse.tile as tile
from concourse import bacc
from concourse.bass_utils import run_bass_kernel_spmd

B, N, C = 2, 2048, 1024
H, D = 16, 64
T = B * N  # 4096 tokens total
NCORES = 8
HPC = H // NCORES  # 2 heads per core
TSL = N // NCORES  # 256-token output slice per core PER BATCH
TF = 512  # projection token-chunk (free dim)
KT = 2 * C // 128  # 16 contraction tiles of 128 over [x_re; x_im]
NCH = N // TF  # token chunks per batch (4)
NKP = N // 256  # key-tile PAIRS per query tile (8): each pair = 2 x 128 keys
F32 = mybir.dt.float32
BF16 = mybir.dt.bfloat16
FP8 = mybir.dt.float8e4
BF = ml_dtypes.bfloat16

# fp8e4m3 exp/v with DoubleRow av+den measured FASTER on paper but fails the
# correctness gate (2.8e-2 rel err: weighted-sum quantization noise does not
# average down) and LDWEIGHTS-bound in practice. Keep the bf16 path.
USE_FP8_AV = False


def _host_prep(inp):
    """Build the host-side sharded/combined arrays (all matmul inputs bf16,
    blocked so every DMA is contiguous per partition)."""
    x_re = np.asarray(inp["x_re"], dtype=np.float32).reshape(T, C)
    x_im = np.asarray(inp["x_im"], dtype=np.float32).reshape(T, C)
    xT2 = np.concatenate([x_re.T, x_im.T], axis=0).astype(BF)  # [2C, T]
    # blocked: [p, chunk, kt, t] so a chunk load is 16KB contiguous/partition
    xb = np.ascontiguousarray(
        xT2.reshape(KT, 128, B * NCH, TF).transpose(1, 2, 0, 3)
    )

    per_core = []
    for c in range(NCORES):
        d = {}
        h0 = c * HPC
        ch = slice(h0 * D, (h0 + HPC) * D)
        for nm in ("q", "k", "v"):
            Wre = np.asarray(inp[f"{nm}_Wre"], dtype=np.float32)[ch]  # [128, C]
            Wim = np.asarray(inp[f"{nm}_Wim"], dtype=np.float32)[ch]
            bre = np.asarray(inp[f"{nm}_bre"], dtype=np.float32)[ch]
            bim = np.asarray(inp[f"{nm}_bim"], dtype=np.float32)[ch]
            Ws, bs = [], []
            for hh in range(HPC):
                hs = slice(hh * D, (hh + 1) * D)
                wr = np.concatenate([Wre[hs].T, -Wim[hs].T], axis=0)  # [2C, 64]
                wi = np.concatenate([Wim[hs].T, Wre[hs].T], axis=0)
                Ws.append(np.concatenate([wr, wi], axis=1))  # [2C, 128]
                bs.append(np.concatenate([bre[hs] - bim[hs], bre[hs] + bim[hs]]))
            if nm == "v":
                wvb = np.concatenate(Ws, axis=1).astype(BF)  # [2C, 256]
                d["wv"] = np.ascontiguousarray(
                    wvb.reshape(KT, 128, 2 * HPC * D).transpose(1, 0, 2)
                )  # [p, kt, 256]
            else:
                wqk = np.stack(Ws).astype(BF)  # [HPC, 2C, 128]
                d[f"w{nm}"] = np.ascontiguousarray(
                    wqk.reshape(HPC, KT, 128, 128).transpose(2, 0, 1, 3)
                )  # [p, hh, kt, m]
                if nm == "q":
                    d["bq"] = np.ascontiguousarray(np.stack(bs, axis=1))  # [128, HPC]
        per_core.append(d)

    # o-projection combined matrices, rows ordered to match the A2A result:
    # src rank r, then per rank [h0:out_r(64), h0:out_i(64), h1:..., h1:...]
    oWre = np.asarray(inp["o_Wre"], dtype=np.float32)
    oWim = np.asarray(inp["o_Wim"], dtype=np.float32)
    Mre_rows, Mim_rows = [], []
    bv_rows = []
    vbre = np.asarray(inp["v_bre"], dtype=np.float32)
    vbim = np.asarray(inp["v_bim"], dtype=np.float32)
    for r in range(NCORES):
        for hh in range(HPC):
            h = r * HPC + hh
            hs = slice(h * D, (h + 1) * D)
            Mre_rows += [oWre[:, hs].T, -oWim[:, hs].T]
            Mim_rows += [oWim[:, hs].T, oWre[:, hs].T]
            bv_rows += [vbre[hs] - vbim[hs], vbre[hs] + vbim[hs]]
    M_re = np.concatenate(Mre_rows, axis=0)  # [2C, C]
    M_im = np.concatenate(Mim_rows, axis=0)
    bv_full = np.concatenate(bv_rows)  # [2C] — v bias in A2A row order
    o_bre = np.asarray(inp["o_bre"], dtype=np.float32)
    o_bim = np.asarray(inp["o_bim"], dtype=np.float32)
    # fold the v bias through the o-projection (softmax rows sum to 1)
    bo_re = (o_bre - o_bim) + M_re.T @ bv_full  # [C]
    bo_im = (o_bre + o_bim) + M_im.T @ bv_full
    bo_re = np.ascontiguousarray(bo_re.reshape(8, 128).T.astype(np.float32))  # [128, 8]
    bo_im = np.ascontiguousarray(bo_im.reshape(8, 128).T.astype(np.float32))

    def mblk(M):  # [2C, C] -> [p, g, kt, 512] contiguous per partition
        Mb = M.astype(BF).reshape(KT, 128, 2, 512)
        return np.ascontiguousarray(Mb.transpose(1, 2, 0, 3))

    shared = dict(
        xb=xb, M_re=mblk(M_re), M_im=mblk(M_im), bo_re=bo_re, bo_im=bo_im
    )
    return shared, per_core


def _build_program():
    nc = bacc.Bacc("TRN2", target_bir_lowering=False, debug=False, num_devices=NCORES)

    # ---- DRAM I/O (host-blocked layouts: contiguous per partition) ----
    xb_d = nc.dram_tensor("xb", [128, B * NCH, KT, TF], BF16, kind="ExternalInput")
    wq_d = nc.dram_tensor("wq", [128, HPC, KT, 128], BF16, kind="ExternalInput")
    wk_d = nc.dram_tensor("wk", [128, HPC, KT, 128], BF16, kind="ExternalInput")
    wv_d = nc.dram_tensor("wv", [128, KT, 2 * HPC * D], BF16, kind="ExternalInput")
    bq_d = nc.dram_tensor("bq", [128, HPC], F32, kind="ExternalInput")
    Mre_d = nc.dram_tensor("M_re", [128, 2, KT, 512], BF16, kind="ExternalInput")
    Mim_d = nc.dram_tensor("M_im", [128, 2, KT, 512], BF16, kind="ExternalInput")
    bore_d = nc.dram_tensor("bo_re", [128, 8], F32, kind="ExternalInput")
    boim_d = nc.dram_tensor("bo_im", [128, 8], F32, kind="ExternalInput")
    # per-core output: rows = [re(1024); im(1024)], cols = [b0 slice | b1 slice]
    yout_d = nc.dram_tensor("yout", [2 * C, B * TSL], F32, kind="ExternalOutput")

    yout_t = yout_d.rearrange("(cb p) t -> p cb t", p=128)  # [128, 16, 512]

    with (
        tile.TileContext(nc) as tc,
        nc.allow_low_precision(
            reason="bf16 intermediates; rounding matches low-precision matmul noise"
        ),
    ):
        with tc.tile_pool(name="dram", bufs=1, space="DRAM") as dram:
            # A2A buffers: [dest rank, ch, TSL tok]; batch 0 whole, b1 per head
            outc_dr0 = dram.tile([NCORES, 256, TSL], BF16, name="outc0", tag="outc0")
            at_dr0 = dram.tile([NCORES, 256, TSL], BF16, name="at0d", tag="at0d")
            outc_dr1 = [
                dram.tile([NCORES, 128, TSL], BF16, name=f"outc1{h}", tag=f"outc1{h}")
                for h in range(HPC)
            ]
            at_dr1 = [
                dram.tile([NCORES, 128, TSL], BF16, name=f"at1{h}", tag=f"at1{h}")
                for h in range(HPC)
            ]

            with (
                tc.tile_pool(name="keep", bufs=1) as keep,
                # PSUM: pair(2 banks x2) + av(1 bank x2) + den(1 bank x2) = 8.
                tc.tile_pool(name="pair_ps", bufs=2, space="PSUM") as pair_ps,
                tc.tile_pool(name="av_ps", bufs=2, space="PSUM") as av_ps,
                tc.tile_pool(name="den_ps", bufs=2, space="PSUM") as den_ps,
                tc.tile_pool(name="qk_sb", bufs=1) as qk_sb,
                tc.tile_pool(name="v_sb", bufs=1) as v_sbp,
                tc.tile_pool(name="expp", bufs=4) as expp,
                tc.tile_pool(name="dtp", bufs=2) as dtp,
                tc.tile_pool(name="evp", bufs=3) as evp,
                tc.tile_pool(name="const", bufs=1) as const,
            ):
                ctx_x = ExitStack()
                xp = ctx_x.enter_context(tc.tile_pool(name="xp", bufs=2))

                # ---- startup: first x chunk split per-2kt across BOTH data
                # rings; weights per-kt on the gpsimd ring so the first chain
                # starts ~2us in ----
                # queue layout: gpsimd = [bq, wq per-kt, wv] so the first q
                # chain's weights and bias land first; x splits ride sync +
                # scalar in parallel; wk follows the scalar x half.
                bq_sb = keep.tile([128, HPC], F32)
                nc.gpsimd.dma_start(bq_sb[:], bq_d[:])
                wq_sb = const.tile([128, HPC, KT, 128], BF16)
                wk_sb = const.tile([128, HPC, KT, 128], BF16)
                wv_sb = const.tile([128, KT, 2 * HPC * D], BF16)
                for kt in range(KT):
                    nc.gpsimd.dma_start(wq_sb[:, :, kt, :], wq_d[:, :, kt, :])
                nc.gpsimd.dma_start(wv_sb[:], wv_d[:])
                xtiles = {}
                xt0 = xp.tile([128, KT, TF], BF16, name="xt", tag="xt")
                xtiles[0] = xt0
                for ks in range(0, KT, 2):
                    eng = nc.sync if (ks // 2) % 2 == 0 else nc.scalar
                    eng.dma_start(xt0[:, ks : ks + 2, :], xb_d[:, 0, ks : ks + 2, :])
                nc.scalar.dma_start(wk_sb[:], wk_d[:])
                if USE_FP8_AV:
                    # all-ones stationary for the DoubleRow denominator matmul
                    ones_t = keep.tile([128, 2, 128], FP8)
                else:
                    ones_t = keep.tile([128, 128], BF16)
                nc.vector.memset(ones_t[:], 1.0)
                # softmax shift (exact): score/8 tops out near +8.1, and
                # fp8e4m3 saturates at 448 — shift so max exp is ~e^4.7
                negone = keep.tile([128, 1], F32)
                nc.vector.memset(negone[:], -3.5)

                qc = {}
                kc = {}
                vt = {}
                for b in range(B):
                    for hh in range(HPC):
                        qc[b, hh] = qk_sb.tile(
                            [128, N], BF16, name=f"qc{b}{hh}", tag=f"qc{b}{hh}"
                        )
                        kc[b, hh] = qk_sb.tile(
                            [128, N], BF16, name=f"kc{b}{hh}", tag=f"kc{b}{hh}"
                        )
                        # fp8: [kp pair, 2, ch] layout feeds DoubleRow directly
                        vt[b, hh] = v_sbp.tile(
                            [128, N // 256, 2, 128] if USE_FP8_AV else [128, N // 128, 128],
                            FP8 if USE_FP8_AV else BF16,
                            name=f"vt{b}{hh}",
                            tag=f"vt{b}{hh}",
                        )

                def emit_x_dma(cg, split=False):
                    """Prefetch x chunk cg (global index). split: per-2kt
                    slices on the sync ring so early chains aren't starved
                    while the weight stream still owns the scalar ring."""
                    xt = xp.tile([128, KT, TF], BF16, name="xt", tag="xt")
                    xtiles[cg] = xt
                    if split:
                        for ks in range(0, KT, 2):
                            nc.sync.dma_start(
                                xt[:, ks : ks + 2, :], xb_d[:, cg, ks : ks + 2, :]
                            )
                    else:
                        eng = nc.scalar if cg % 2 == 0 else nc.sync
                        eng.dma_start(xt[:], xb_d[:, cg, :, :])

                def emit_proj_qk(b, ci):
                    """q and k chains for chunk ci of batch b."""
                    cg = b * NCH + ci
                    xt = xtiles[cg]
                    csl = slice(ci * TF, ci * TF + TF)
                    prs = {}
                    for hh in range(HPC):
                        prs[hh] = pair_ps.tile([128, 1024], F32, name="prps", tag="prps")
                        for kt in range(KT):
                            nc.tensor.matmul(
                                prs[hh][:, 0:512],
                                wq_sb[:, hh, kt, :],
                                xt[:, kt, :],
                                start=(kt == 0),
                                stop=(kt == KT - 1),
                            )
                        nc.scalar.activation(
                            qc[b, hh][:, csl],
                            prs[hh][:, 0:512],
                            mybir.ActivationFunctionType.Identity,
                            bias=bq_sb[:, hh : hh + 1],
                        )
                    for hh in range(HPC):
                        for kt in range(KT):
                            nc.tensor.matmul(
                                prs[hh][:, 512:1024],
                                wk_sb[:, hh, kt, :],
                                xt[:, kt, :],
                                start=(kt == 0),
                                stop=(kt == KT - 1),
                            )
                        # split the drains across DVE and the (idle) scalar
                        # engine so the PSUM frees sooner
                        if hh == 0:
                            nc.vector.tensor_copy(kc[b, hh][:, csl], prs[hh][:, 512:1024])
                        else:
                            nc.scalar.copy(kc[b, hh][:, csl], prs[hh][:, 512:1024])

                def emit_proj_v(b, ci, borrow=False):
                    """v chains for chunk ci of batch b. borrow: attention is
                    not running, so rotate through the av/den PSUM banks too
                    (doubles the drain lead time ahead of each chain)."""
                    cg = b * NCH + ci
                    xt = xtiles[cg]
                    if borrow:
                        vps = [
                            av_ps.tile([128, 512], F32, name="avps", tag="avps")[:, 0:256],
                            den_ps.tile([128, 512], F32, name="denps", tag="denps")[:, 0:256],
                            av_ps.tile([128, 512], F32, name="avps", tag="avps")[:, 0:256],
                            den_ps.tile([128, 512], F32, name="denps", tag="denps")[:, 0:256],
                        ]
                    else:
                        vtiles = [
                            pair_ps.tile([128, 1024], F32, name="prps", tag="prps")
                            for _ in range(2)
                        ]
                        vps = [
                            vtiles[m % 2][:, (m // 2) * 512 : (m // 2) * 512 + 256]
                            for m in range(4)
                        ]
                    for m in range(TF // 128):
                        vp = vps[m]
                        for kt in range(KT):
                            nc.tensor.matmul(
                                vp,
                                xt[:, kt, m * 128 : (m + 1) * 128],
                                wv_sb[:, kt, :],
                                start=(kt == 0),
                                stop=(kt == KT - 1),
                            )
                        ktok = ci * (TF // 128) + m
                        for hh in range(HPC):
                            dst = (
                                vt[b, hh][:, ktok // 2, ktok % 2, :]
                                if USE_FP8_AV
                                else vt[b, hh][:, ktok, :]
                            )
                            if hh == 0:
                                nc.vector.tensor_copy(
                                    dst, vp[:, hh * 128 : (hh + 1) * 128]
                                )
                            else:
                                nc.scalar.copy(dst, vp[:, hh * 128 : (hh + 1) * 128])

                deferred = [None]  # single-slot pipeline for softmax tails

                def flush_deferred():
                    if deferred[0] is not None:
                        deferred[0]()
                        deferred[0] = None

                def emit_attn_qt(b, hh, qt, filler=None):
                    """Scores/exp/av for one 512-query tile; the softmax tail
                    (den matmuls, reciprocal, scale, store) is deferred into
                    the next tile so the PE never idles at tile boundaries.
                    filler: independent PE work emitted mid-tile (kp==3),
                    where the PSUM rotation has maximum slack."""
                    qsl = slice(qt * 512, (qt + 1) * 512)
                    av = av_ps.tile([128, 512], F32, name="avps", tag="avps")

                    def _av(kp, ex):
                        for j in range(2):
                            kt = 2 * kp + j
                            nc.tensor.matmul(
                                av[:],
                                vt[b, hh][:, kt, :],
                                ex[:, j, :],
                                start=(kt == 0),
                                stop=(kt == N // 128 - 1),
                            )

                    pend = []
                    exs = []
                    pairs = {}
                    quads = {}
                    for kp in range(NKP):
                        pr = pair_ps.tile([128, 1024], F32, name="prps", tag="prps")
                        for j in range(2):
                            kt = 2 * kp + j
                            nc.tensor.matmul(
                                pr[:, j * 512 : (j + 1) * 512],
                                kc[b, hh][:, kt * 128 : (kt + 1) * 128],
                                qc[b, hh][:, qsl],
                                start=True,
                                stop=True,
                            )
                        ex = expp.tile([128, 2, 512], BF16, name="ex", tag="ex")
                        nc.scalar.activation(
                            ex[:, :, :],
                            pr[:],
                            mybir.ActivationFunctionType.Exp,
                            scale=0.125,
                        )
                        exs.append(ex)
                        # denominator tree-adds on the idle DVE (bf16 2x mode)
                        if kp % 2 == 1:
                            p = kp // 2
                            pairs[p] = dtp.tile(
                                [128, 1024], BF16, name=f"dp{p % 2}", tag=f"dp{p % 2}"
                            )
                            nc.vector.tensor_tensor(
                                pairs[p][:], exs[kp - 1][:], ex[:], mybir.AluOpType.add
                            )
                        if kp % 4 == 3:
                            q4 = kp // 4
                            quads[q4] = dtp.tile(
                                [128, 1024], BF16, name=f"dq{q4}", tag=f"dq{q4}"
                            )
                            nc.vector.tensor_tensor(
                                quads[q4][:],
                                pairs[kp // 2 - 1][:],
                                pairs[kp // 2][:],
                                mybir.AluOpType.add,
                            )
                        pend.append((kp, ex))
                        if kp == 1:
                            # PE is 2 key-pairs into this tile: safe point to
                            # emit the previous tile's softmax tail
                            flush_deferred()
                        if kp == 3 and filler is not None:
                            filler()
                        if len(pend) > 2:
                            _av(*pend.pop(0))
                    pend_tail = list(pend)

                    def tail():
                        # last two av accumulations land here so the PE never
                        # waits on the exp pipeline at the tile boundary
                        for item in pend_tail:
                            _av(*item)
                        den = den_ps.tile([128, 512], F32, name="denps", tag="denps")
                        for i4, q4t in enumerate((quads[0], quads[1])):
                            for j in range(2):
                                nc.tensor.matmul(
                                    den[:],
                                    ones_t[:],
                                    q4t[:, j * 512 : (j + 1) * 512],
                                    start=(i4 == 0 and j == 0),
                                    stop=(i4 == 1 and j == 1),
                                )
                        rb = evp.tile([128, 512], F32, name="rb", tag="rb")
                        nc.vector.reciprocal_approx_fast(rb[:], den[:])
                        outc = evp.tile([128, 512], BF16, name="outc", tag="outc")
                        nc.vector.tensor_tensor(
                            outc[:], av[:], rb[:], mybir.AluOpType.mult
                        )
                        for j in range(2):
                            if b == 0:
                                dst = outc_dr0[2 * qt + j, hh * 128 : (hh + 1) * 128, :]
                            else:
                                dst = outc_dr1[hh][2 * qt + j, :, :]
                            nc.sync.dma_start(dst, outc[:, j * TSL : (j + 1) * TSL])

                    deferred[0] = tail

                # ---- phase 1: projections for batch 0. Chunk prefetches are
                # emitted AFTER the consuming chains of the previous chunk so
                # they never jump the DMA line ahead of the weight stream ----
                for ci in range(NCH):
                    emit_proj_qk(0, ci)
                    if ci + 1 < NCH:
                        emit_x_dma(ci + 1, split=(ci == 0))
                    emit_proj_v(0, ci, borrow=True)
                emit_x_dma(NCH)  # first b1 chunk

                # ---- phase 2a: attention(b0) with HALF of proj(b1) woven in
                # (fills the exp-bound slack and keeps the PE at full clock);
                # the other half runs after the b0 A2A fires, hiding the
                # first collective's cross-core skew sync (~17us) ----
                def _f(si):
                    if si == 0:
                        return lambda: emit_proj_qk(1, 0)
                    if si == 1:
                        return lambda: (emit_x_dma(NCH + 1), emit_proj_v(1, 0))
                    if si == 2:
                        return lambda: emit_proj_qk(1, 1)
                    if si == 3:
                        return lambda: (emit_x_dma(NCH + 2), emit_proj_v(1, 1))
                    if si == 5:
                        return lambda: emit_x_dma(NCH + 3)
                    return None

                for hh in range(HPC):
                    for qt in range(N // 512):
                        emit_attn_qt(0, hh, qt, filler=_f(hh * 4 + qt))
                flush_deferred()
                nc.gpsimd.collective_compute(
                    "AllToAll",
                    mybir.AluOpType.bypass,
                    replica_groups=[list(range(NCORES))],
                    ins=[outc_dr0.opt()],
                    outs=[at_dr0.opt()],
                )

                # ---- phase 2b: rest of proj(b1) under the b0 exchange ----
                for ci in (2, 3):
                    emit_proj_qk(1, ci)
                    emit_proj_v(1, ci, borrow=True)

                # x pool no longer needed: free it for the o-phase M tiles
                ctx_x.close()
                opool = ctx_x.enter_context(tc.tile_pool(name="opool", bufs=1))
                oev = ctx_x.enter_context(tc.tile_pool(name="oev", bufs=2))
                m_tiles = []
                for g in range(2):
                    for part, M_d in ((0, Mre_d), (1, Mim_d)):
                        m_sb = opool.tile(
                            [128, KT, 512], BF16, name=f"m{g}{part}", tag=f"m{g}{part}"
                        )
                        eng = nc.scalar if part == 0 else nc.sync
                        eng.dma_start(m_sb[:], M_d[:, g, :, :])
                        m_tiles.append((g, part, m_sb))
                bo_sb = [None, None]
                bo_sb[0] = keep.tile([128, 8], F32, name="bo_re", tag="bo_re")
                bo_sb[1] = keep.tile([128, 8], F32, name="bo_im", tag="bo_im")
                nc.gpsimd.dma_start(bo_sb[0][:], bore_d[:])
                nc.gpsimd.dma_start(bo_sb[1][:], boim_d[:])
                at_sb = [None, None]
                at_sb[0] = opool.tile(
                    [128, HPC, NCORES, TSL], BF16, name="at0", tag="at0"
                )
                at_sb[1] = opool.tile(
                    [128, HPC, NCORES, TSL], BF16, name="at1", tag="at1"
                )
                # at0 load on the gpsimd queue right after the A2A it waits on
                at0_t = at_dr0.rearrange("r (hp p) t -> p hp r t", p=128)
                for h in range(HPC):  # DMA APs are limited to 3 dims
                    nc.gpsimd.dma_start(at_sb[0][:, h, :, :], at0_t[:, h, :, :])

                def emit_oproj_group(b, g, part, m_sb):
                    """One quarter of the o-projection for batch b: 4 chains
                    of 128 output channels x TSL tokens + bias + store."""
                    otiles = [
                        pair_ps.tile([128, 1024], F32, name="prps", tag="prps"),
                        pair_ps.tile([128, 1024], F32, name="prps", tag="prps"),
                    ]
                    y_sb = oev.tile([128, 4, TSL], F32, name="y_sb", tag="y_sb")
                    for i in range(4):
                        # alternate tiles/half-banks: ACT reads chain i-1's
                        # bank while the PE accumulates into another
                        ps = otiles[i % 2][:, (i // 2) * 512 : (i // 2) * 512 + TSL]
                        for kt in range(KT):
                            # contraction row kt = (src rank kt//2, head kt%2)
                            nc.tensor.matmul(
                                ps,
                                m_sb[:, kt, i * 128 : (i + 1) * 128],
                                at_sb[b][:, kt % 2, kt // 2, :],
                                start=(kt == 0),
                                stop=(kt == KT - 1),
                            )
                        nc.scalar.activation(
                            y_sb[:, i, :],
                            ps,
                            mybir.ActivationFunctionType.Identity,
                            bias=bo_sb[part][:, g * 4 + i : g * 4 + i + 1],
                        )
                    cb0 = part * 8 + g * 4
                    nc.sync.dma_start(
                        yout_t[:, cb0 : cb0 + 4, b * TSL : (b + 1) * TSL], y_sb[:]
                    )

                # ---- phase 3: attention(b1) interleaved with o-proj(b0).
                # Groups go after mid-head qts only: a group after the last
                # qt of a head would delay that head's outc stores and A2A ----
                og = []
                for hh in range(HPC):
                    for qt in range(N // 512):
                        emit_attn_qt(1, hh, qt)
                        if (hh, qt) in og:
                            gi = og.index((hh, qt))
                            emit_oproj_group(0, *m_tiles[gi][:2], m_tiles[gi][2])
                    flush_deferred()
                    # head hh's rows ship while the next head computes; its
                    # at-load queues right behind the exchange on gpsimd
                    nc.gpsimd.collective_compute(
                        "AllToAll",
                        mybir.AluOpType.bypass,
                        replica_groups=[list(range(NCORES))],
                        ins=[outc_dr1[hh].opt()],
                        outs=[at_dr1[hh].opt()],
                    )
                    at1h_t = at_dr1[hh].rearrange("r p t -> p r t")
                    nc.gpsimd.dma_start(at_sb[1][:, hh, :, :], at1h_t[:, :, :])

                # ---- phase 4: o-proj(b0) under the last A2A ----
                for gi in (0, 1, 2, 3):
                    emit_oproj_group(0, *m_tiles[gi][:2], m_tiles[gi][2])
                for gi in range(4):
                    emit_oproj_group(1, *m_tiles[gi][:2], m_tiles[gi][2])
                ctx_x.close()  # opool/oev close before the outer pools (LIFO)
    nc.compile()
    return nc


_NC_CACHE = None


def _get_program():
    global _NC_CACHE
    if _NC_CACHE is None:
        _NC_CACHE = _build_program()
    return _NC_CACHE


def _run(inputs, trace=False, trace_kwargs=None):
    shared, per_core = _host_prep(inputs)
    nc = _get_program()
    in_maps = []
    for c in range(NCORES):
        d = per_core[c]
        in_maps.append(
            {
                "xb": shared["xb"],
                "wq": d["wq"],
                "wk": d["wk"],
                "wv": d["wv"],
                "bq": d["bq"],
                "M_re": shared["M_re"],
                "M_im": shared["M_im"],
                "bo_re": shared["bo_re"],
                "bo_im": shared["bo_im"],
            }
        )
    res = run_bass_kernel_spmd(
        nc, in_maps, list(range(NCORES)), trace=trace, **(trace_kwargs or {})
    )
    youts = [res.results[c]["yout"] for c in range(NCORES)]
    # youts[c]: [2C, B*TSL]; rows [re(1024); im(1024)], cols [b0 256 | b1 256]
    re = np.zeros((B, N, C), dtype=np.float32)
    im = np.zeros((B, N, C), dtype=np.float32)
    for c in range(NCORES):
        for b in range(B):
            tsl = slice(c * TSL, (c + 1) * TSL)
            re[b, tsl] = youts[c][:C, b * TSL : (b + 1) * TSL].T
            im[b, tsl] = youts[c][C:, b * TSL : (b + 1) * TSL].T
    return np.stack([re, im]).astype(np.float32), res


def kernel(**inputs) -> np.ndarray:
    out, _ = _run(inputs, trace=False)
    return out
